# revision 17
# baseline (speedup 1.0000x reference)
"""Multi-head attention TRN2 kernel (b=4, n=2048, e=768, h=8 heads, d=96).

Sharding: 8 cores = 4 batches x 2 head-groups (4 heads each).
Each core computes, for its (batch, head-group):
    qkv projection (its heads' columns of Wqkv), per-head attention
    (softmax over full n=2048), and a partial output projection
    (its heads' rows of Wproj). Host sums the two partial outputs per
    batch (row-parallel linear unshard) and concatenates batches.

All matmul operands are bf16 (fp32 PSUM accumulation), which keeps the PE
at 1 col/cycle while halving LDWEIGHTS time (FWL enabled for 128-col
stationaries) and DMA traffic. Scores are computed transposed (ET[nk, nq])
so no transposes are needed; softmax denominators come from an extra
ones-column appended to V (row 96 of the PV accumulator). exp() skips
max-subtraction: logits/sqrt(e) here are bounded (~|2|). Per-head
normalized outputs stay in SBUF and are restacked to a K=128-packed
[3 x 128, n] layout with SBUF->SBUF DMAs; the output projection then runs
3 matmuls per chunk. PV matmuls run LAG kb-steps behind exp so the
previous pass's normalization (which reads the single PSUM accumulator)
overlaps the next pass's first score/exp steps. QK projections for head
h+1 are spread evenly across both of head h's attention passes; the
output projection for the first 8 row blocks interleaves into the last
attention pass.
"""

import os

import numpy as np

import concourse.bacc as bacc
import concourse.mybir as mybir
import concourse.tile as tile
from concourse.bass_utils import run_bass_kernel_spmd

B, N, E = 4, 2048, 768
H = 8          # total heads
HL = 4         # heads per core
D = E // H     # 96
DH = D + 1     # 97 (with denominator column)
KB = E // 128  # 6 contraction blocks
NB = N // 128  # 16 row blocks
NC = 8         # cores
EL = HL * D    # 384 local e-dim
SCALE = float(E) ** -0.5
LAG = 3        # PV runs LAG kb-steps behind exp

F32 = mybir.dt.float32
BF16 = mybir.dt.bfloat16
F8 = mybir.dt.float8e4
DR = mybir.MatmulPerfMode.DoubleRow
# 'bf16': plain bf16 scores (best error, PE col rate is 1/cycle regardless).
# 'dr_resid': fp8 DoubleRow, q as (hi, lo) residual pair — same speed as bf16.
# 'dc_plain': fp8 DoubleColumn, q/k plain fp8 — measured same speed as bf16.
SCORE_MODE = os.environ.get("SCORE_MODE", "bf16")
SCORE_PERF = {
    "bf16": None,
    "dr_resid": DR,
    "dc_plain": mybir.MatmulPerfMode.DoubleColumn,
}[SCORE_MODE]
QKDT = BF16 if SCORE_MODE == "bf16" else F8
AF = mybir.ActivationFunctionType
MULT = mybir.AluOpType.mult
ADD = mybir.AluOpType.add
SUB = mybir.AluOpType.subtract

_COMPILED = None
LAST_EXEC_NS = None
LAST_RESULTS = None


def _device_reset():
    """Recover a wedged NeuronCore (NRT_EXEC_UNIT_UNRECOVERABLE) via axon."""
    try:
        import ctypes
        import time

        import jax

        jax.devices()
        lib = ctypes.CDLL("/opt/axon/libaxon_pjrt.so")
        lib.axon_reset.restype = ctypes.c_int64
        lib.axon_reset()
        time.sleep(3)
    except Exception:
        pass


def _build():
    nc = bacc.Bacc("TRN2", target_bir_lowering=False, debug=False)

    xT_d = nc.dram_tensor("xT", [E, N], BF16, kind="ExternalInput")
    wq_d = nc.dram_tensor("wq", [E, EL], BF16, kind="ExternalInput")
    wk_d = nc.dram_tensor("wk", [E, EL], BF16, kind="ExternalInput")
    wv_d = nc.dram_tensor("wv", [E, HL * DH], BF16, kind="ExternalInput")
    bq_d = nc.dram_tensor("bq", [D, HL], F32, kind="ExternalInput")
    bk_d = nc.dram_tensor("bk", [D, HL], F32, kind="ExternalInput")
    bv_d = nc.dram_tensor("bv", [1, HL * DH], BF16, kind="ExternalInput")
    wp_d = nc.dram_tensor("wp", [EL, E], BF16, kind="ExternalInput")
    bp_d = nc.dram_tensor("bp", [1, E], BF16, kind="ExternalInput")
    ones_d = nc.dram_tensor("ones", [1, 128], BF16, kind="ExternalInput")
    ones32_d = nc.dram_tensor("ones32", [1, 128], F32, kind="ExternalInput")
    out_d = nc.dram_tensor("out", [N, E], BF16, kind="ExternalOutput")

    with tile.TileContext(nc) as tc:
        with (
            tc.tile_pool(name="const", bufs=1) as cpool,
            tc.tile_pool(name="xt", bufs=1) as xpool,
            tc.tile_pool(name="qk", bufs=2) as qkpool,
            tc.tile_pool(name="vh", bufs=1) as vpool,
            tc.tile_pool(name="pt", bufs=LAG + 3) as ptpool,
            tc.tile_pool(name="nrm", bufs=3) as npool,
            tc.tile_pool(name="on", bufs=1) as opool,
            tc.tile_pool(name="pp", bufs=2, space="PSUM") as pp,
            tc.tile_pool(name="pattn", bufs=1, space="PSUM") as pattn,
        ):
            # ---- constants (DMA order matters: vproj prereqs first) ----
            ones_sb = cpool.tile([1, 128], BF16, tag="ones")
            nc.sync.dma_start(ones_sb[:], ones_d[:])
            ones32_sb = cpool.tile([1, 128], F32, tag="ones32")
            nc.sync.dma_start(ones32_sb[:], ones32_d[:])
            bv_sb = cpool.tile([1, HL * DH], BF16, tag="bv")
            nc.sync.dma_start(bv_sb[:], bv_d[:])

            # xT first-column chunks, spread across three queues so all six
            # are in flight within ~1us (gpsimd descriptor issue is slow).
            xT_sb = []
            for kb in range(KB):
                t = xpool.tile([128, N], BF16, tag=f"xt{kb}", name=f"xt{kb}")
                xT_sb.append(t)
            for kb, eng in zip(
                range(KB),
                [nc.gpsimd, nc.gpsimd, nc.sync, nc.sync, nc.scalar, nc.scalar],
            ):
                eng.dma_start(
                    xT_sb[kb][:, 0:512], xT_d[kb * 128:(kb + 1) * 128, 0:512]
                )

            # load the exp ACT table set off the critical path
            scr = npool.tile([1, 16], F32, tag="scr", bufs=1)
            nc.scalar.activation(scr[:], ones_sb[:, 0:16], AF.Exp)

            # wv rides the scalar queue: ACT is idle during the ramp, and this
            # lets xT (gpsimd), wv (scalar) and wq/wk (sync) stream in parallel
            wv_sb = []
            for kb in range(KB):
                t = cpool.tile([128, HL * DH], BF16, tag=f"wv{kb}")
                nc.scalar.dma_start(t[:], wv_d[kb * 128:(kb + 1) * 128, :])
                wv_sb.append(t)
            bq_sb = cpool.tile([D, HL], F32, tag="bq")
            nc.sync.dma_start(bq_sb[:], bq_d[:])
            bk_sb = cpool.tile([D, HL], F32, tag="bk")
            nc.sync.dma_start(bk_sb[:], bk_d[:])
            wq_sb = []
            wk_sb = []
            for kb in range(KB):
                t = cpool.tile([128, EL], BF16, tag=f"wq{kb}")
                nc.sync.dma_start(t[:], wq_d[kb * 128:(kb + 1) * 128, :])
                wq_sb.append(t)
                t = cpool.tile([128, EL], BF16, tag=f"wk{kb}")
                nc.sync.dma_start(t[:], wk_d[kb * 128:(kb + 1) * 128, :])
                wk_sb.append(t)
            for c in range(1, 4):
                for kb in range(KB):
                    nc.gpsimd.dma_start(
                        xT_sb[kb][:, c * 512:(c + 1) * 512],
                        xT_d[kb * 128:(kb + 1) * 128, c * 512:(c + 1) * 512],
                    )
            # wp loads are deferred into the head-1 attention pass: they are
            # not needed until the output projection (~80% through) and would
            # steal startup HBM bandwidth from xT/wq/wk.
            wp_sb = []

            def load_wp():
                for g in range(3):
                    t = cpool.tile([128, E], BF16, tag=f"wp{g}")
                    nc.gpsimd.dma_start(t[:], wp_d[g * 128:(g + 1) * 128, :])
                    wp_sb.append(t)

            bp_sb = cpool.tile([1, E], BF16, tag="bp")
            nc.sync.dma_start(bp_sb[:], bp_d[:])

            # broadcast bias tiles (one K=1 matmul each, reused everywhere)
            bvb_sb = cpool.tile([128, HL * DH], BF16, tag="bvb")
            ps = pp.tile([128, 512], F32, tag="pp")
            nc.tensor.matmul(ps[:, 0:HL * DH], ones_sb[:], bv_sb[:], start=True, stop=True)
            nc.vector.tensor_copy(bvb_sb[:], ps[:, 0:HL * DH])
            bpb_sb = cpool.tile([128, E], BF16, tag="bpb")

            # normalized per-head outputs, heads stacked along partitions:
            # otn[qh][g][128g + r, i] = OT_(r//96)[r % 96, qh*1024 + i]
            otn = [
                [
                    opool.tile(
                        [128, 1024], BF16, tag=f"otn{qh}_{g}", name=f"otn{qh}_{g}"
                    )
                    for g in range(3)
                ]
                for qh in range(2)
            ]

            def start_qkproj(h):
                # q is kept as an fp8 (hi, lo) residual pair so the DoubleRow
                # scores matmul sees q at ~bf16 precision; k is quantized once
                # to fp8 (its pair dim is a stride-0 broadcast in the matmul).
                with nc.named_scope(f"qkproj{h}"):
                    nq = 1 if SCORE_MODE != "dr_resid" else 2
                    qT = qkpool.tile([D, nq, N], QKDT, tag="qT", name=f"qT{h}")
                    kT = qkpool.tile([D, N], QKDT, tag="kT", name=f"kT{h}")
                return (qT, kT)

            def emit_qkproj_chunk(h, tiles, i):
                qT, kT = tiles
                qk, c = divmod(i, 4)
                w_sb, b_sb = [(wq_sb, bq_sb), (wk_sb, bk_sb)][qk]
                sl = slice(c * 512, (c + 1) * 512)
                with nc.named_scope(f"qkproj{h}"):
                    ps = pp.tile([128, 512], F32, tag="pp", name=f"psqk{h}_{i}")
                    for kb in range(KB):
                        nc.tensor.matmul(
                            ps[0:D, :],
                            w_sb[kb][:, h * D:(h + 1) * D],
                            xT_sb[kb][:, sl],
                            start=(kb == 0),
                            stop=(kb == KB - 1),
                        )
                    if qk == 1:
                        nc.vector.tensor_scalar(
                            kT[:, sl], ps[0:D, :], 1.0, b_sb[:, h:h + 1], MULT, ADD
                        )
                    elif SCORE_MODE == "dr_resid":
                        qtmp = npool.tile([D, 512], BF16, tag="qtmp", bufs=2)
                        nc.vector.tensor_scalar(
                            qtmp[:], ps[0:D, :], 1.0, b_sb[:, h:h + 1], MULT, ADD
                        )
                        nc.vector.tensor_copy(qT[:, 0, sl], qtmp[:])
                        nc.vector.tensor_tensor(
                            qT[:, 1, sl], qtmp[:], qT[:, 0, sl], SUB
                        )
                    else:
                        nc.vector.tensor_scalar(
                            qT[:, 0, sl], ps[0:D, :], 1.0, b_sb[:, h:h + 1],
                            MULT, ADD,
                        )

            # ---- V-hat projection interleaved with head-0 qk projection ----
            vhat = []
            tiles = start_qkproj(0)
            for c in range(4):
                with nc.named_scope("vproj"):
                    for nb in range(4 * c, 4 * c + 4):
                        ps = pp.tile([128, 512], F32, tag="pp")
                        for kb in range(KB):
                            nc.tensor.matmul(
                                ps[:, 0:HL * DH],
                                xT_sb[kb][:, nb * 128:(nb + 1) * 128],
                                wv_sb[kb][:],
                                start=(kb == 0),
                                stop=(kb == KB - 1),
                            )
                        vt = vpool.tile([128, HL * DH], BF16, tag=f"vh{nb}")
                        nc.vector.tensor_tensor(vt[:], ps[:, 0:HL * DH], bvb_sb[:], ADD)
                        vhat.append(vt)
                emit_qkproj_chunk(0, tiles, c)       # q chunk c
                emit_qkproj_chunk(0, tiles, 4 + c)   # k chunk c
                if c == 1:
                    # build the bproj broadcast (off the critical start)
                    for off, w in [(0, 512), (512, 256)]:
                        ps = pp.tile([128, 512], F32, tag="pp")
                        nc.tensor.matmul(
                            ps[:, 0:w], ones_sb[:], bp_sb[:, off:off + w],
                            start=True, stop=True,
                        )
                        nc.vector.tensor_copy(bpb_sb[:, off:off + w], ps[:, 0:w])

            def drain_norm(h, qh, acc, tail=False):
                """Drain the PSUM accumulator (bf16 rows + f32 denominator
                row) so the next pass's PV(0) gets the buffer back fast, and
                run the reciprocal chains (DVE/ACT only — no PE-queue ops).
                Mid-kernel: batched ops, sums on ACT (lands in the boundary
                ACT hole before exp(0)). Tail: per-j chains with ACT offload
                so the j=0 half unblocks the output projection early."""
                with nc.named_scope(f"norm{h}_{qh}"):
                    if not tail:
                        a = npool.tile([D, 1024], BF16, tag="acc_bf")
                        nc.vector.tensor_copy(a[:], acc[0:D, :])
                        sums = npool.tile([1, 1024], F32, tag="sums")
                        nc.scalar.copy(sums[:], acc[D:DH, :])
                        rec = npool.tile([1, 1024], F32, tag="rec")
                        nc.vector.reciprocal_approx_fast(rec[:], sums[:])
                        recb = npool.tile([1, 1024], BF16, tag="recb")
                        nc.vector.tensor_copy(recb[:], rec[:])
                        accbf = [a[:, 0:512], a[:, 512:1024]]
                        recbs = [recb[:, 0:512], recb[:, 512:1024]]
                        return (h, qh, accbf, recbs)
                    accbf, recbs = [], []
                    for j in range(2):
                        sl = slice(j * 512, (j + 1) * 512)
                        sums = npool.tile([1, 512], F32, tag="sumst")
                        if j == 0:
                            nc.vector.tensor_copy(sums[:], acc[D:DH, sl])
                        else:
                            nc.scalar.copy(sums[:], acc[D:DH, sl])
                        rec = npool.tile([1, 512], F32, tag="rect")
                        nc.vector.reciprocal_approx_fast(rec[:], sums[:])
                        recb = npool.tile([1, 512], BF16, tag="recbt")
                        if j == 0:
                            nc.vector.tensor_copy(recb[:], rec[:])
                        else:
                            nc.scalar.copy(recb[:], rec[:])
                        a = npool.tile([D, 512], BF16, tag="acc_bft")
                        nc.scalar.copy(a[:], acc[0:D, sl])
                        accbf.append(a[:])
                        recbs.append(recb[:])
                return (h, qh, accbf, recbs)

            def finish_norm(h, qh, accbf, recbs):
                """Broadcast the reciprocals, scale, and stack into the
                K=128-packed otn layout. Emitted a few kb into the following
                pass so the bc matmuls never stall the PE queue."""
                with nc.named_scope(f"norm{h}_{qh}"):
                    for j in range(2):
                        sl = slice(j * 512, (j + 1) * 512)
                        bc = pp.tile([128, 512], F32, tag="pp")
                        nc.tensor.matmul(
                            bc[0:D, :], ones_sb[:, 0:D], recbs[j],
                            start=True, stop=True,
                        )
                        if h == 0:
                            # rows 0..95 land on the same partitions: write the
                            # stack tile directly, no shift DMA needed
                            nc.vector.tensor_tensor(
                                otn[qh][0][0:D, sl], accbf[j], bc[0:D, :], MULT
                            )
                            continue
                        ot = npool.tile([D, 512], BF16, tag="ot", bufs=4)
                        nc.vector.tensor_tensor(ot[:], accbf[j], bc[0:D, :], MULT)
                        # stack rows 96h..96h+95 into the K=128-packed layout
                        r0 = D * h
                        g0, off = divmod(r0, 128)
                        n0 = min(128 - off, D)
                        nc.sync.dma_start(otn[qh][g0][off:off + n0, sl], ot[0:n0, :])
                        if n0 < D:
                            nc.sync.dma_start(
                                otn[qh][g0 + 1][0:D - n0, sl], ot[n0:D, :]
                            )

            def emit_out(nb, slot):
                qh, col = divmod(nb, 8)
                col *= 128
                with nc.named_scope(f"out{nb}"):
                    if slot == "pp":
                        pa = pp.tile([128, 512], F32, tag="pp")
                        pb = pp.tile([128, 512], F32, tag="pp")
                        A, Bv = pa[:, 0:512], pb[:, 0:256]
                    elif slot == "acc":
                        t = pattn.tile([128, 1024], F32, tag="acc", bufs=1)
                        A, Bv = t[:, 0:512], t[:, 512:768]
                    else:
                        t = pattn.tile([128, 1024], F32, tag="et", bufs=2)
                        A, Bv = t[:, 0:512], t[:, 512:768]
                    for g in range(3):
                        nc.tensor.matmul(
                            A,
                            otn[qh][g][:, col:col + 128],
                            wp_sb[g][:, 0:512],
                            start=(g == 0),
                            stop=(g == 2),
                        )
                    for g in range(3):
                        nc.tensor.matmul(
                            Bv,
                            otn[qh][g][:, col:col + 128],
                            wp_sb[g][:, 512:768],
                            start=(g == 0),
                            stop=(g == 2),
                        )
                    osb = npool.tile([128, E], BF16, tag="osb", bufs=4)
                    nc.vector.tensor_tensor(osb[:, 0:512], A, bpb_sb[:, 0:512], ADD)
                    nc.vector.tensor_tensor(osb[:, 512:768], Bv, bpb_sb[:, 512:768], ADD)
                    nc.gpsimd.dma_start(out_d[nb * 128:(nb + 1) * 128, :], osb[:])

            # ---- per-head attention; qkproj of h+1 spread over both passes ----
            pending_norm = None
            for h in range(HL):
                qT, kT = tiles
                nxt = None
                for qh in range(2):
                    with nc.named_scope(f"attn{h}_{qh}"):
                        acc = pattn.tile([128, 1024], F32, tag="acc")

                        def emit_pv(kbp, pt):
                            for j in range(2):
                                nc.tensor.matmul(
                                    acc[0:DH, j * 512:(j + 1) * 512],
                                    vhat[kbp][:, h * DH:(h + 1) * DH],
                                    pt[:, j * 512:(j + 1) * 512],
                                    start=(kbp == 0),
                                    stop=(kbp == NB - 1),
                                )

                        pts = []
                        for kb in range(NB):
                            et = pattn.tile([128, 1024], F32, tag="et", bufs=2)
                            if SCORE_MODE == "dr_resid":
                                kTb = kT[:, kb * 128:(kb + 1) * 128].unsqueeze(
                                    1
                                ).broadcast_to((D, 2, 128))
                            else:
                                kTb = kT[:, kb * 128:(kb + 1) * 128]
                            for j in range(2):
                                c = 2 * qh + j
                                if SCORE_MODE == "dr_resid":
                                    rhs = qT[:, :, c * 512:(c + 1) * 512]
                                else:
                                    rhs = qT[:, 0, c * 512:(c + 1) * 512]
                                nc.tensor.matmul(
                                    et[:, j * 512:(j + 1) * 512],
                                    kTb,
                                    rhs,
                                    start=True,
                                    stop=True,
                                    perf_mode=SCORE_PERF,
                                )
                            if kb >= LAG:
                                emit_pv(kb - LAG, pts[kb - LAG])
                            pt = ptpool.tile([128, 1024], BF16, tag="pt")
                            nc.scalar.activation(pt[:], et[:], AF.Exp, scale=SCALE)
                            pts.append(pt)
                            if kb == LAG and pending_norm is not None:
                                # previous pass's normalization: by now its
                                # reciprocal chain is done, so the broadcast
                                # matmuls won't stall the PE queue
                                finish_norm(*pending_norm)
                                pending_norm = None
                            if h == 1 and qh == 0 and kb == 6:
                                load_wp()
                            g = qh * 16 + kb
                            if h + 1 < HL and g % 4 == 0:
                                if g == 0:
                                    nxt = start_qkproj(h + 1)
                                emit_qkproj_chunk(h + 1, nxt, g // 4)
                            if h == 3 and qh == 1 and kb >= 5 and kb % 2 == 1:
                                emit_out((kb - 5) // 2, "pp")
                        for kbp in range(NB - LAG, NB):
                            emit_pv(kbp, pts[kbp])
                        pending_norm = drain_norm(h, qh, acc, tail=(h == HL - 1 and qh == 1))
                tiles = nxt

            # ---- output projection tail ----
            # nb 6/7 only need the qh=0 stacks: they keep the PE busy while
            # the last norm's reciprocal chain runs (finish between them so
            # its mults aren't queued behind both blocks' DVE adds)
            emit_out(6, "et")
            finish_norm(*pending_norm)
            emit_out(7, "pp")
            for nb, slot in zip(range(8, NB), ("et", "pp", "acc", "et", "pp", "acc", "et", "pp")):
                emit_out(nb, slot)

    nc.compile()
    return nc


def _shard(x, Wqkv, bqkv, Wproj, bproj):
    """Build per-core input maps. Core c -> (batch c//2, head-group c%2)."""
    import ml_dtypes

    bf16 = ml_dtypes.bfloat16
    Wr = np.ascontiguousarray(Wqkv.reshape(E, H, D, 3))
    br = np.ascontiguousarray(bqkv.reshape(H, D, 3))
    ones = np.ones((1, 128), bf16)
    in_maps = []
    for c in range(NC):
        bb, hg = divmod(c, 2)
        hs = slice(hg * HL, (hg + 1) * HL)
        wq = np.ascontiguousarray(Wr[:, hs, :, 0].reshape(E, EL)).astype(bf16)
        wk = np.ascontiguousarray(Wr[:, hs, :, 1].reshape(E, EL)).astype(bf16)
        wv = np.zeros((E, HL, DH), np.float32)
        wv[:, :, :D] = Wr[:, hs, :, 2]
        bq = np.ascontiguousarray(br[hs, :, 0].T)  # [D, HL] (scale applied at exp)
        bk = np.ascontiguousarray(br[hs, :, 1].T)
        bv = np.zeros((HL, DH), np.float32)
        bv[:, :D] = br[hs, :, 2]
        bv[:, D] = 1.0  # denominator ones column
        wp = np.ascontiguousarray(Wproj[hg * EL:(hg + 1) * EL, :]).astype(bf16)
        bp = bproj if hg == 0 else np.zeros_like(bproj)
        in_maps.append({
            "xT": np.ascontiguousarray(x[bb].T).astype(bf16),
            "wq": wq,
            "wk": wk,
            "wv": np.ascontiguousarray(wv.reshape(E, HL * DH)).astype(bf16),
            "bq": np.ascontiguousarray(bq, dtype=np.float32),
            "bk": np.ascontiguousarray(bk, dtype=np.float32),
            "bv": np.ascontiguousarray(bv.reshape(1, HL * DH)).astype(bf16),
            "wp": wp,
            "bp": np.ascontiguousarray(bp.reshape(1, E)).astype(bf16),
            "ones": ones,
            "ones32": np.ones((1, 128), np.float32),
        })
    return in_maps


def kernel(x, Wqkv, bqkv, Wproj, bproj):
    global _COMPILED, LAST_EXEC_NS, LAST_RESULTS
    x = np.asarray(x, dtype=np.float32)
    Wqkv = np.asarray(Wqkv, dtype=np.float32)
    bqkv = np.asarray(bqkv, dtype=np.float32)
    Wproj = np.asarray(Wproj, dtype=np.float32)
    bproj = np.asarray(bproj, dtype=np.float32)

    if _COMPILED is None:
        _COMPILED = _build()
    nc = _COMPILED

    in_maps = _shard(x, Wqkv, bqkv, Wproj, bproj)
    trace = bool(int(os.environ.get("BASS_MHA_TRACE", "0")))
    try:
        res = run_bass_kernel_spmd(nc, in_maps, list(range(NC)), trace=trace)
    except Exception:
        _device_reset()
        res = run_bass_kernel_spmd(nc, in_maps, list(range(NC)), trace=trace)
    LAST_EXEC_NS = res.exec_time_ns
    LAST_RESULTS = res

    out = np.empty((B, N, E), np.float32)
    for bb in range(B):
        out[bb] = res.results[2 * bb]["out"].astype(np.float32) + res.results[
            2 * bb + 1
        ]["out"].astype(np.float32)
    return out



# revision 63
# speedup vs baseline: 1.0691x; 1.0691x over previous
"""Multi-head attention TRN2 kernel (b=4, n=2048, e=768, h=8 heads, d=96).

Sharding: 8 cores = 4 batches x 2 head-groups (4 heads each).
Each core computes, for its (batch, head-group):
    qkv projection (its heads' columns of Wqkv), per-head attention
    (softmax over full n=2048), and a partial output projection
    (its heads' rows of Wproj). Host sums the two bf16 partial outputs
    per batch in f32 (row-parallel linear unshard) and concatenates.

All matmul operands are bf16 (fp32 PSUM accumulation): the TRN2 PE
streams exactly one moving column per cycle for every dtype (fp8
DoubleRow/DoubleColumn only extend the contraction depth, measured on
hw), so bf16 is already at peak column rate and fp8 would only add
quantization error. Scores are computed transposed (ET[nk, nq]) so no
transposes are needed; 1/sqrt(e) is folded into the exp activation's
scale; softmax denominators come from an extra ones-column appended to V
(row 96 of the PV accumulator). exp() skips max-subtraction: scaled
logits are bounded (~|2|). Per-head normalized outputs stay in SBUF and
are restacked to a K=128-packed [3 x 128, n] layout with SBUF->SBUF
DMAs; the output projection then runs 3 matmuls per chunk.

Schedule notes (all measured on hw):
- PV matmuls run LAG kb-steps behind exp; the LAG-deep PV tail covers
  the ACT exp backlog at pass boundaries. The norm drain uses DVE only
  (an ACT op there delays the next pass's exps and starves the PE of
  et buffers).
- The startup is HBM-bandwidth/latency-bound: only vproj c0-c2 and
  q0/q1/k0/k1 of head 0 run up front; head-0's q2/q3/k2/k3 defer into
  attn0_0. wp loads defer to mid-kernel; wk rides gpsimd behind xT c1.
- h3's k1-k3 chunks are emitted in its own first pass (bunched at kb
  0-2 to cover the boundary backlog); the remaining h3 chunks spread
  3+2 over h2's passes.
- out blocks 0-5 interleave into the last pass (stacks ready early via
  finish_norm at kb3); the tail rotates out8-15 over four PSUM slots.
"""

import os

import numpy as np

import concourse.bacc as bacc
import concourse.mybir as mybir
import concourse.tile as tile
from concourse.bass_utils import run_bass_kernel_spmd

B, N, E = 4, 2048, 768
H = 8          # total heads
HL = 4         # heads per core
D = E // H     # 96
DH = D + 1     # 97 (with denominator column)
KB = E // 128  # 6 contraction blocks
NB = N // 128  # 16 row blocks
NC = 8         # cores
EL = HL * D    # 384 local e-dim
SCALE = float(E) ** -0.5
LAG = 5        # PV runs LAG kb-steps behind exp

F32 = mybir.dt.float32
BF16 = mybir.dt.bfloat16
F8 = mybir.dt.float8e4
DR = mybir.MatmulPerfMode.DoubleRow
# 'bf16': plain bf16 scores (best error, PE col rate is 1/cycle regardless).
# 'dr_resid': fp8 DoubleRow, q as (hi, lo) residual pair — same speed as bf16.
# 'dc_plain': fp8 DoubleColumn, q/k plain fp8 — measured same speed as bf16.
SCORE_MODE = os.environ.get("SCORE_MODE", "bf16")
SCORE_PERF = {
    "bf16": None,
    "dr_resid": DR,
    "dc_plain": mybir.MatmulPerfMode.DoubleColumn,
}[SCORE_MODE]
QKDT = BF16 if SCORE_MODE == "bf16" else F8
AF = mybir.ActivationFunctionType
MULT = mybir.AluOpType.mult
ADD = mybir.AluOpType.add
SUB = mybir.AluOpType.subtract

_COMPILED = None
LAST_EXEC_NS = None
LAST_RESULTS = None


def _device_reset():
    """Recover a wedged NeuronCore (NRT_EXEC_UNIT_UNRECOVERABLE) via axon."""
    try:
        import ctypes
        import time

        import jax

        jax.devices()
        lib = ctypes.CDLL("/opt/axon/libaxon_pjrt.so")
        lib.axon_reset.restype = ctypes.c_int64
        lib.axon_reset()
        time.sleep(3)
    except Exception:
        pass


def _build():
    nc = bacc.Bacc("TRN2", target_bir_lowering=False, debug=False)

    xT_d = nc.dram_tensor("xT", [E, N], BF16, kind="ExternalInput")
    wq_d = nc.dram_tensor("wq", [E, EL], BF16, kind="ExternalInput")
    wk_d = nc.dram_tensor("wk", [E, EL], BF16, kind="ExternalInput")
    wv_d = nc.dram_tensor("wv", [E, HL * DH], BF16, kind="ExternalInput")
    bq_d = nc.dram_tensor("bq", [D, HL], F32, kind="ExternalInput")
    bk_d = nc.dram_tensor("bk", [D, HL], F32, kind="ExternalInput")
    bv_d = nc.dram_tensor("bv", [1, HL * DH], BF16, kind="ExternalInput")
    wp_d = nc.dram_tensor("wp", [EL, E], BF16, kind="ExternalInput")
    bp_d = nc.dram_tensor("bp", [1, E], BF16, kind="ExternalInput")
    ones_d = nc.dram_tensor("ones", [1, 128], BF16, kind="ExternalInput")
    ones32_d = nc.dram_tensor("ones32", [1, 128], F32, kind="ExternalInput")
    out_d = nc.dram_tensor("out", [N, E], BF16, kind="ExternalOutput")

    with tile.TileContext(nc) as tc:
        with (
            tc.tile_pool(name="const", bufs=1) as cpool,
            tc.tile_pool(name="xt", bufs=1) as xpool,
            tc.tile_pool(name="qk", bufs=2) as qkpool,
            tc.tile_pool(name="vh", bufs=1) as vpool,
            tc.tile_pool(name="pt", bufs=LAG + 3) as ptpool,
            tc.tile_pool(name="nrm", bufs=3) as npool,
            tc.tile_pool(name="on", bufs=1) as opool,
            tc.tile_pool(name="pp", bufs=2, space="PSUM") as pp,
            tc.tile_pool(name="pattn", bufs=1, space="PSUM") as pattn,
        ):
            # ---- constants (DMA order matters: vproj prereqs first) ----
            ones_sb = cpool.tile([1, 128], BF16, tag="ones")
            nc.sync.dma_start(ones_sb[:], ones_d[:])
            bv_sb = cpool.tile([1, HL * DH], BF16, tag="bv")
            nc.sync.dma_start(bv_sb[:], bv_d[:])

            # xT first-column chunks, spread across three queues so all six
            # are in flight within ~1us (gpsimd descriptor issue is slow).
            xT_sb = []
            for kb in range(KB):
                t = xpool.tile([128, N], BF16, tag=f"xt{kb}", name=f"xt{kb}")
                xT_sb.append(t)
            for kb, eng in zip(
                range(KB),
                [nc.gpsimd, nc.gpsimd, nc.sync, nc.sync, nc.scalar, nc.scalar],
            ):
                eng.dma_start(
                    xT_sb[kb][:, 0:512], xT_d[kb * 128:(kb + 1) * 128, 0:512]
                )

            # wv split across scalar+sync so the last tile lands sooner (the
            # first vproj block needs all six)
            wv_sb = []
            for kb in range(KB):
                t = cpool.tile([128, HL * DH], BF16, tag=f"wv{kb}")
                eng = nc.scalar if kb < 3 else nc.sync
                eng.dma_start(t[:], wv_d[kb * 128:(kb + 1) * 128, :])
                wv_sb.append(t)

            # load the exp ACT table set off the critical path
            scr = npool.tile([1, 16], F32, tag="scr", bufs=1)
            nc.scalar.activation(scr[:], ones_sb[:, 0:16], AF.Exp)
            bq_sb = cpool.tile([D, HL], F32, tag="bq")
            nc.sync.dma_start(bq_sb[:], bq_d[:])
            bk_sb = cpool.tile([D, HL], F32, tag="bk")
            nc.sync.dma_start(bk_sb[:], bk_d[:])
            bp_sb = cpool.tile([1, E], BF16, tag="bp")
            nc.sync.dma_start(bp_sb[:], bp_d[:])

            def xt_dma(eng, kb, c):
                eng.dma_start(
                    xT_sb[kb][:, c * 512:(c + 1) * 512],
                    xT_d[kb * 128:(kb + 1) * 128, c * 512:(c + 1) * 512],
                )

            # wq on sync (q-chunks run first and need only xT c0 + wq); wk on
            # gpsimd AFTER xT c1 so its 0.59MB doesn't compete for HBM during
            # the bandwidth-bound first ~8us (k0 isn't consumed until ~+16us)
            wq_sb = []
            wk_sb = []
            for kb in range(KB):
                t = cpool.tile([128, EL], BF16, tag=f"wq{kb}")
                nc.sync.dma_start(t[:], wq_d[kb * 128:(kb + 1) * 128, :])
                wq_sb.append(t)
            for kb in range(KB):
                xt_dma(nc.gpsimd, kb, 1)
            for kb in range(KB):
                t = cpool.tile([128, EL], BF16, tag=f"wk{kb}")
                nc.gpsimd.dma_start(t[:], wk_d[kb * 128:(kb + 1) * 128, :])
                wk_sb.append(t)
            for c in (2, 3):
                for kb in range(KB):
                    xt_dma(nc.gpsimd, kb, c)
            # wp loads are deferred into the head-1 attention pass: they are
            # not needed until the output projection (~80% through) and would
            # steal startup HBM bandwidth from xT/wq/wk.
            wp_sb = []

            def load_wp():
                for g in range(3):
                    t = cpool.tile([128, E], BF16, tag=f"wp{g}")
                    nc.gpsimd.dma_start(t[:], wp_d[g * 128:(g + 1) * 128, :])
                    wp_sb.append(t)

            # broadcast bias tiles (one K=1 matmul each, reused everywhere)
            bvb_sb = cpool.tile([128, HL * DH], BF16, tag="bvb")
            ps = pp.tile([128, 512], F32, tag="pp")
            nc.tensor.matmul(ps[:, 0:HL * DH], ones_sb[:], bv_sb[:], start=True, stop=True)
            nc.vector.tensor_copy(bvb_sb[:], ps[:, 0:HL * DH])
            bpb_sb = cpool.tile([128, E], BF16, tag="bpb")

            # normalized per-head outputs, heads stacked along partitions:
            # otn[qh][g][128g + r, i] = OT_(r//96)[r % 96, qh*1024 + i]
            otn = [
                [
                    opool.tile(
                        [128, 1024], BF16, tag=f"otn{qh}_{g}", name=f"otn{qh}_{g}"
                    )
                    for g in range(3)
                ]
                for qh in range(2)
            ]

            def start_qkproj(h):
                # q is kept as an fp8 (hi, lo) residual pair so the DoubleRow
                # scores matmul sees q at ~bf16 precision; k is quantized once
                # to fp8 (its pair dim is a stride-0 broadcast in the matmul).
                with nc.named_scope(f"qkproj{h}"):
                    nq = 1 if SCORE_MODE != "dr_resid" else 2
                    qT = qkpool.tile([D, nq, N], QKDT, tag="qT", name=f"qT{h}")
                    kT = qkpool.tile([D, N], QKDT, tag="kT", name=f"kT{h}")
                return (qT, kT)

            def emit_qkproj_chunk(h, tiles, i):
                qT, kT = tiles
                qk, c = divmod(i, 4)
                w_sb, b_sb = [(wq_sb, bq_sb), (wk_sb, bk_sb)][qk]
                sl = slice(c * 512, (c + 1) * 512)
                with nc.named_scope(f"qkproj{h}"):
                    ps = pp.tile([128, 512], F32, tag="pp", name=f"psqk{h}_{i}")
                    for kb in range(KB):
                        nc.tensor.matmul(
                            ps[0:D, :],
                            w_sb[kb][:, h * D:(h + 1) * D],
                            xT_sb[kb][:, sl],
                            start=(kb == 0),
                            stop=(kb == KB - 1),
                        )
                    if qk == 1:
                        nc.vector.tensor_scalar(
                            kT[:, sl], ps[0:D, :], 1.0, b_sb[:, h:h + 1], MULT, ADD
                        )
                    elif SCORE_MODE == "dr_resid":
                        qtmp = npool.tile([D, 512], BF16, tag="qtmp", bufs=2)
                        nc.vector.tensor_scalar(
                            qtmp[:], ps[0:D, :], 1.0, b_sb[:, h:h + 1], MULT, ADD
                        )
                        nc.vector.tensor_copy(qT[:, 0, sl], qtmp[:])
                        nc.vector.tensor_tensor(
                            qT[:, 1, sl], qtmp[:], qT[:, 0, sl], SUB
                        )
                    else:
                        nc.vector.tensor_scalar(
                            qT[:, 0, sl], ps[0:D, :], 1.0, b_sb[:, h:h + 1],
                            MULT, ADD,
                        )

            # ---- V-hat projection interleaved with head-0 qk projection ----
            # Only vproj c0-c2, q0, q1 and k0 run up front: the startup region
            # is HBM-bandwidth-bound, so the rest of head-0's chunks and the
            # vproj c3 block group are deferred into the attn0_0 pass.
            vhat = []

            def emit_vproj_block(nb):
                with nc.named_scope("vproj"):
                    ps = pp.tile([128, 512], F32, tag="pp")
                    for kb in range(KB):
                        nc.tensor.matmul(
                            ps[:, 0:HL * DH],
                            xT_sb[kb][:, nb * 128:(nb + 1) * 128],
                            wv_sb[kb][:],
                            start=(kb == 0),
                            stop=(kb == KB - 1),
                        )
                    vt = vpool.tile([128, HL * DH], BF16, tag=f"vh{nb}")
                    nc.vector.tensor_tensor(vt[:], ps[:, 0:HL * DH], bvb_sb[:], ADD)
                    vhat.append(vt)

            tiles = start_qkproj(0)
            for c in range(4):
                for nb in range(4 * c, 4 * c + 4):
                    emit_vproj_block(nb)
                # q0, q1, k0, k1 up front; q2/q3/k2/k3 defer into attn0_0
                emit_qkproj_chunk(0, tiles, [0, 1, 4, 5][c])
                if c == 1:
                    # build the bproj broadcast (off the critical start)
                    for off, w in [(0, 512), (512, 256)]:
                        ps = pp.tile([128, 512], F32, tag="pp")
                        nc.tensor.matmul(
                            ps[:, 0:w], ones_sb[:], bp_sb[:, off:off + w],
                            start=True, stop=True,
                        )
                        nc.vector.tensor_copy(bpb_sb[:, off:off + w], ps[:, 0:w])

            def drain_norm(h, qh, acc, tail=False):
                """Drain the PSUM accumulator (bf16 rows + f32 denominator
                row) so the next pass's PV(0) gets the buffer back fast, and
                run the reciprocal chains (DVE only — ACT still has an exp
                backlog at the pass boundary, so any ACT op here delays the
                next pass's exps and starves the PE of et buffers). Tail:
                per-j chains with ACT offload so the j=0 half unblocks the
                output projection early."""
                with nc.named_scope(f"norm{h}_{qh}"):
                    if not tail:
                        a = npool.tile([D, 1024], BF16, tag="acc_bf")
                        nc.vector.tensor_copy(a[:], acc[0:D, :])
                        sums = npool.tile([1, 1024], F32, tag="sums")
                        nc.vector.tensor_copy(sums[:], acc[D:DH, :])
                        rec = npool.tile([1, 1024], F32, tag="rec")
                        nc.vector.reciprocal_approx_fast(rec[:], sums[:])
                        recb = npool.tile([1, 1024], BF16, tag="recb")
                        nc.vector.tensor_copy(recb[:], rec[:])
                        accbf = [a[:, 0:512], a[:, 512:1024]]
                        recbs = [recb[:, 0:512], recb[:, 512:1024]]
                        return (h, qh, accbf, recbs)
                    accbf, recbs = [], []
                    for j in range(2):
                        sl = slice(j * 512, (j + 1) * 512)
                        sums = npool.tile([1, 512], F32, tag="sumst")
                        if j == 0:
                            nc.vector.tensor_copy(sums[:], acc[D:DH, sl])
                        else:
                            nc.scalar.copy(sums[:], acc[D:DH, sl])
                        rec = npool.tile([1, 512], F32, tag="rect")
                        nc.vector.reciprocal_approx_fast(rec[:], sums[:])
                        recb = npool.tile([1, 512], BF16, tag="recbt")
                        if j == 0:
                            nc.vector.tensor_copy(recb[:], rec[:])
                        else:
                            nc.scalar.copy(recb[:], rec[:])
                        a = npool.tile([D, 512], BF16, tag="acc_bft")
                        nc.scalar.copy(a[:], acc[0:D, sl])
                        accbf.append(a[:])
                        recbs.append(recb[:])
                return (h, qh, accbf, recbs)

            def finish_norm(h, qh, accbf, recbs):
                """Broadcast the reciprocals, scale, and stack into the
                K=128-packed otn layout. Emitted a few kb into the following
                pass so the bc matmuls never stall the PE queue."""
                with nc.named_scope(f"norm{h}_{qh}"):
                    for j in range(2):
                        sl = slice(j * 512, (j + 1) * 512)
                        bc = pp.tile([128, 512], F32, tag="pp")
                        nc.tensor.matmul(
                            bc[0:D, :], ones_sb[:, 0:D], recbs[j],
                            start=True, stop=True,
                        )
                        if h == 0:
                            # rows 0..95 land on the same partitions: write the
                            # stack tile directly, no shift DMA needed
                            nc.vector.tensor_tensor(
                                otn[qh][0][0:D, sl], accbf[j], bc[0:D, :], MULT
                            )
                            continue
                        ot = npool.tile([D, 512], BF16, tag="ot", bufs=4)
                        nc.vector.tensor_tensor(ot[:], accbf[j], bc[0:D, :], MULT)
                        # stack rows 96h..96h+95 into the K=128-packed layout
                        # (DMAs alternate queues so the tail stacks land fast)
                        r0 = D * h
                        g0, off = divmod(r0, 128)
                        n0 = min(128 - off, D)
                        e1, e2 = (nc.sync, nc.scalar) if j == 0 else (nc.scalar, nc.sync)
                        e1.dma_start(otn[qh][g0][off:off + n0, sl], ot[0:n0, :])
                        if n0 < D:
                            e2.dma_start(
                                otn[qh][g0 + 1][0:D - n0, sl], ot[n0:D, :]
                            )

            def emit_out(nb, slot):
                qh, col = divmod(nb, 8)
                col *= 128
                with nc.named_scope(f"out{nb}"):
                    if slot == "pp":
                        pa = pp.tile([128, 512], F32, tag="pp")
                        pb = pp.tile([128, 512], F32, tag="pp")
                        A, Bv = pa[:, 0:512], pb[:, 0:256]
                    elif slot == "acc":
                        t = pattn.tile([128, 1024], F32, tag="acc", bufs=1)
                        A, Bv = t[:, 0:512], t[:, 512:768]
                    else:
                        t = pattn.tile([128, 1024], F32, tag="et", bufs=2)
                        A, Bv = t[:, 0:512], t[:, 512:768]
                    for g in range(3):
                        nc.tensor.matmul(
                            A,
                            otn[qh][g][:, col:col + 128],
                            wp_sb[g][:, 0:512],
                            start=(g == 0),
                            stop=(g == 2),
                        )
                    for g in range(3):
                        nc.tensor.matmul(
                            Bv,
                            otn[qh][g][:, col:col + 128],
                            wp_sb[g][:, 512:768],
                            start=(g == 0),
                            stop=(g == 2),
                        )
                    osb = npool.tile([128, E], BF16, tag="osb", bufs=4)
                    nc.vector.tensor_tensor(osb[:, 0:512], A, bpb_sb[:, 0:512], ADD)
                    nc.vector.tensor_tensor(osb[:, 512:768], Bv, bpb_sb[:, 512:768], ADD)
                    # spread across three queues so the tail transfers drain
                    # in parallel (exec time runs to the last DMA completion)
                    eng = [nc.sync, nc.scalar, nc.gpsimd][nb % 3]
                    eng.dma_start(out_d[nb * 128:(nb + 1) * 128, :], osb[:])

            # ---- per-head attention; qkproj of h+1 spread over both passes ----
            pending_norm = None
            for h in range(HL):
                qT, kT = tiles
                nxt = None
                for qh in range(2):
                    with nc.named_scope(f"attn{h}_{qh}"):
                        acc = pattn.tile([128, 1024], F32, tag="acc")

                        def emit_pv(kbp, pt):
                            for j in range(2):
                                nc.tensor.matmul(
                                    acc[0:DH, j * 512:(j + 1) * 512],
                                    vhat[kbp][:, h * DH:(h + 1) * DH],
                                    pt[:, j * 512:(j + 1) * 512],
                                    start=(kbp == 0),
                                    stop=(kbp == NB - 1),
                                )

                        pts = []
                        for kb in range(NB):
                            # exp stays one [128, 1024] op: splitting it into
                            # halves costs ~80ns/op of ACT overhead (+19us of
                            # exp time overall — measured regression)
                            et = pattn.tile([128, 1024], F32, tag="et", bufs=2)
                            if SCORE_MODE == "dr_resid":
                                kTb = kT[:, kb * 128:(kb + 1) * 128].unsqueeze(
                                    1
                                ).broadcast_to((D, 2, 128))
                            else:
                                kTb = kT[:, kb * 128:(kb + 1) * 128]
                            for j in range(2):
                                c = 2 * qh + j
                                if SCORE_MODE == "dr_resid":
                                    rhs = qT[:, :, c * 512:(c + 1) * 512]
                                else:
                                    rhs = qT[:, 0, c * 512:(c + 1) * 512]
                                nc.tensor.matmul(
                                    et[:, j * 512:(j + 1) * 512],
                                    kTb,
                                    rhs,
                                    start=True,
                                    stop=True,
                                    perf_mode=SCORE_PERF,
                                )
                            if kb >= LAG:
                                emit_pv(kb - LAG, pts[kb - LAG])
                            pt = ptpool.tile([128, 1024], BF16, tag="pt")
                            nc.scalar.activation(pt[:], et[:], AF.Exp, scale=SCALE)
                            pts.append(pt)
                            fin_kb = 3 if (h == 3 and qh == 1) else LAG
                            if kb == fin_kb and pending_norm is not None:
                                # previous pass's normalization: by now its
                                # reciprocal chain is done, so the broadcast
                                # matmuls won't stall the PE queue. One step
                                # earlier in the last pass so the otn stacks
                                # land before the first interleaved out block.
                                finish_norm(*pending_norm)
                                pending_norm = None
                            if h == 1 and qh == 0 and kb == 6:
                                load_wp()
                            g = qh * 16 + kb
                            if h + 1 < HL and g % 4 == 0:
                                # h3's k1..k3 chunks are deferred into its own
                                # first pass (which has no next-head projection
                                # and would idle); the remaining five h3 chunks
                                # spread 3+2 over h2's PE-bound passes.
                                if g == 0:
                                    nxt = start_qkproj(h + 1)
                                if h == 2:
                                    idx = {0: 0, 4: 1, 8: 2, 16: 3, 20: 4}.get(g)
                                    if idx is not None:
                                        emit_qkproj_chunk(3, nxt, idx)
                                else:
                                    emit_qkproj_chunk(h + 1, nxt, g // 4)
                            if h == 3 and qh == 0 and kb in (0, 1, 2):
                                # bunched at kb0-2: covers the PE idle window
                                # while ACT drains the previous pass's exps
                                emit_qkproj_chunk(3, (qT, kT), 5 + kb)
                            if h == 0 and qh == 0 and kb in (1, 5, 9, 11):
                                # head-0 work deferred out of the bw-bound
                                # startup: k2/k3 ready well before their
                                # scores (kb 8/12), q2/q3 before attn0_1
                                emit_qkproj_chunk(
                                    0, (qT, kT), {1: 6, 5: 7, 9: 2, 11: 3}[kb]
                                )
                            if h == 3 and qh == 1 and kb >= 5 and kb % 2 == 1:
                                emit_out((kb - 5) // 2, "pp")
                        for kbp in range(NB - LAG, NB):
                            emit_pv(kbp, pts[kbp])
                        pending_norm = drain_norm(h, qh, acc, tail=(h == HL - 1 and qh == 1))
                tiles = nxt

            # ---- output projection tail ----
            # nb 6/7 only need the qh=0 stacks: they keep the PE busy while
            # the last norm's reciprocal chain runs
            emit_out(6, "pp")
            finish_norm(*pending_norm)
            emit_out(7, "et")
            # 4-deep PSUM slot rotation (et alternates its two buffers) so
            # slot recycling isn't gated on the tail's busy DVE
            for nb, slot in zip(range(8, NB), ("acc", "et", "pp", "et", "acc", "et", "pp", "et")):
                emit_out(nb, slot)

    nc.compile()
    return nc


def _shard(x, Wqkv, bqkv, Wproj, bproj):
    """Build per-core input maps. Core c -> (batch c//2, head-group c%2)."""
    import ml_dtypes

    bf16 = ml_dtypes.bfloat16
    Wr = np.ascontiguousarray(Wqkv.reshape(E, H, D, 3))
    br = np.ascontiguousarray(bqkv.reshape(H, D, 3))
    ones = np.ones((1, 128), bf16)
    in_maps = []
    for c in range(NC):
        bb, hg = divmod(c, 2)
        hs = slice(hg * HL, (hg + 1) * HL)
        wq = np.ascontiguousarray(Wr[:, hs, :, 0].reshape(E, EL)).astype(bf16)
        wk = np.ascontiguousarray(Wr[:, hs, :, 1].reshape(E, EL)).astype(bf16)
        wv = np.zeros((E, HL, DH), np.float32)
        wv[:, :, :D] = Wr[:, hs, :, 2]
        bq = np.ascontiguousarray(br[hs, :, 0].T)  # [D, HL] (scale applied at exp)
        bk = np.ascontiguousarray(br[hs, :, 1].T)
        bv = np.zeros((HL, DH), np.float32)
        bv[:, :D] = br[hs, :, 2]
        bv[:, D] = 1.0  # denominator ones column
        wp = np.ascontiguousarray(Wproj[hg * EL:(hg + 1) * EL, :]).astype(bf16)
        bp = bproj if hg == 0 else np.zeros_like(bproj)
        in_maps.append({
            "xT": np.ascontiguousarray(x[bb].T).astype(bf16),
            "wq": wq,
            "wk": wk,
            "wv": np.ascontiguousarray(wv.reshape(E, HL * DH)).astype(bf16),
            "bq": np.ascontiguousarray(bq, dtype=np.float32),
            "bk": np.ascontiguousarray(bk, dtype=np.float32),
            "bv": np.ascontiguousarray(bv.reshape(1, HL * DH)).astype(bf16),
            "wp": wp,
            "bp": np.ascontiguousarray(bp.reshape(1, E)).astype(bf16),
            "ones": ones,
            "ones32": np.ones((1, 128), np.float32),
        })
    return in_maps


def kernel(x, Wqkv, bqkv, Wproj, bproj):
    global _COMPILED, LAST_EXEC_NS, LAST_RESULTS
    x = np.asarray(x, dtype=np.float32)
    Wqkv = np.asarray(Wqkv, dtype=np.float32)
    bqkv = np.asarray(bqkv, dtype=np.float32)
    Wproj = np.asarray(Wproj, dtype=np.float32)
    bproj = np.asarray(bproj, dtype=np.float32)

    if _COMPILED is None:
        _COMPILED = _build()
    nc = _COMPILED

    in_maps = _shard(x, Wqkv, bqkv, Wproj, bproj)
    trace = bool(int(os.environ.get("BASS_MHA_TRACE", "0")))
    try:
        res = run_bass_kernel_spmd(nc, in_maps, list(range(NC)), trace=trace)
    except Exception:
        _device_reset()
        res = run_bass_kernel_spmd(nc, in_maps, list(range(NC)), trace=trace)
    LAST_EXEC_NS = res.exec_time_ns
    LAST_RESULTS = res

    out = np.empty((B, N, E), np.float32)
    for bb in range(B):
        out[bb] = res.results[2 * bb]["out"].astype(np.float32) + res.results[
            2 * bb + 1
        ]["out"].astype(np.float32)
    return out



# revision 76
# speedup vs baseline: 1.2827x; 1.1998x over previous
"""Multi-head attention TRN2 kernel (b=4, n=2048, e=768, h=8 heads, d=96).

Sharding: 8 cores = 4 batches x 2 head-groups (4 heads each).
Each core computes, for its (batch, head-group):
    qkv projection (its heads' columns of Wqkv), per-head attention
    (softmax over full n=2048), and a partial output projection
    (its heads' rows of Wproj). Host sums the two bf16 partial outputs
    per batch in f32 (row-parallel linear unshard) and concatenates.

All matmul operands are bf16 (fp32 PSUM accumulation): the TRN2 PE
streams exactly one moving column per cycle for every dtype (fp8
DoubleRow/DoubleColumn only extend the contraction depth, measured on
hw), so bf16 is already at peak column rate and fp8 would only add
quantization error. Scores are computed transposed (ET[nk, nq]) so no
transposes are needed; 1/sqrt(e) is folded into the exp activation's
scale; softmax denominators come from an extra ones-column appended to V
(row 96 of the PV accumulator). exp() skips max-subtraction: scaled
logits are bounded (~|2|). Per-head normalized outputs stay in SBUF and
are restacked to a K=128-packed [3 x 128, n] layout with SBUF->SBUF
DMAs; the output projection then runs 3 matmuls per chunk.

Schedule notes (all measured on hw):
- PV matmuls run LAG kb-steps behind exp; the LAG-deep PV tail covers
  the ACT exp backlog at pass boundaries. The norm drain uses DVE only
  (an ACT op there delays the next pass's exps and starves the PE of
  et buffers).
- The startup is HBM-bandwidth/latency-bound: only vproj c0-c2 and
  q0/q1/k0/k1 of head 0 run up front; head-0's q2/q3/k2/k3 defer into
  attn0_0. wp loads defer to mid-kernel; wk rides gpsimd behind xT c1.
- h3's k1-k3 chunks are emitted in its own first pass (bunched at kb
  0-2 to cover the boundary backlog); the remaining h3 chunks spread
  3+2 over h2's passes.
- out blocks 0-5 interleave into the last pass (stacks ready early via
  finish_norm at kb3); the tail rotates out8-15 over four PSUM slots.
"""

import os

import numpy as np

import concourse.bacc as bacc
import concourse.mybir as mybir
import concourse.tile as tile
from concourse.bass_utils import run_bass_kernel_spmd

B, N, E = 4, 2048, 768
H = 8          # total heads
HL = 4         # heads per core
D = E // H     # 96
DH = D + 1     # 97 (with denominator column)
KB = E // 128  # 6 contraction blocks
NB = N // 128  # 16 row blocks
NC = 8         # cores
EL = HL * D    # 384 local e-dim
SCALE = float(E) ** -0.5
LAG = 5        # PV runs LAG kb-steps behind exp

F32 = mybir.dt.float32
BF16 = mybir.dt.bfloat16
F8 = mybir.dt.float8e4
DR = mybir.MatmulPerfMode.DoubleRow
# 'bf16': plain bf16 scores (best error, PE col rate is 1/cycle regardless).
# 'dr_resid': fp8 DoubleRow, q as (hi, lo) residual pair — same speed as bf16.
# 'dc_plain': fp8 DoubleColumn, q/k plain fp8 — measured same speed as bf16.
SCORE_MODE = os.environ.get("SCORE_MODE", "bf16")
SCORE_PERF = {
    "bf16": None,
    "dr_resid": DR,
    "dc_plain": mybir.MatmulPerfMode.DoubleColumn,
}[SCORE_MODE]
QKDT = BF16 if SCORE_MODE == "bf16" else F8
AF = mybir.ActivationFunctionType
MULT = mybir.AluOpType.mult
ADD = mybir.AluOpType.add
SUB = mybir.AluOpType.subtract

_COMPILED = None
LAST_EXEC_NS = None
LAST_RESULTS = None


def _device_reset():
    """Recover a wedged NeuronCore (NRT_EXEC_UNIT_UNRECOVERABLE) via axon."""
    try:
        import ctypes
        import time

        import jax

        jax.devices()
        lib = ctypes.CDLL("/opt/axon/libaxon_pjrt.so")
        lib.axon_reset.restype = ctypes.c_int64
        lib.axon_reset()
        time.sleep(3)
    except Exception:
        pass


def _build():
    nc = bacc.Bacc("TRN2", target_bir_lowering=False, debug=False)

    xT_d = nc.dram_tensor("xT", [E, N], BF16, kind="ExternalInput")
    wq_d = nc.dram_tensor("wq", [E, EL], BF16, kind="ExternalInput")
    wk_d = nc.dram_tensor("wk", [E, EL], BF16, kind="ExternalInput")
    wv_d = nc.dram_tensor("wv", [E, HL * DH], BF16, kind="ExternalInput")
    bq_d = nc.dram_tensor("bq", [D, HL], F32, kind="ExternalInput")
    bk_d = nc.dram_tensor("bk", [D, HL], F32, kind="ExternalInput")
    bv_d = nc.dram_tensor("bv", [1, HL * DH], BF16, kind="ExternalInput")
    wp_d = nc.dram_tensor("wp", [EL, E], BF16, kind="ExternalInput")
    bp_d = nc.dram_tensor("bp", [1, E], BF16, kind="ExternalInput")
    ones_d = nc.dram_tensor("ones", [1, 128], BF16, kind="ExternalInput")
    ones32_d = nc.dram_tensor("ones32", [1, 128], F32, kind="ExternalInput")
    out_d = nc.dram_tensor("out", [N, E], BF16, kind="ExternalOutput")

    with tile.TileContext(nc) as tc:
        with (
            tc.tile_pool(name="const", bufs=1) as cpool,
            tc.tile_pool(name="xt", bufs=1) as xpool,
            tc.tile_pool(name="qk", bufs=2) as qkpool,
            tc.tile_pool(name="vh", bufs=1) as vpool,
            tc.tile_pool(name="pt", bufs=LAG + 3) as ptpool,
            tc.tile_pool(name="nrm", bufs=3) as npool,
            tc.tile_pool(name="on", bufs=1) as opool,
            tc.tile_pool(name="pp", bufs=2, space="PSUM") as pp,
            tc.tile_pool(name="pattn", bufs=1, space="PSUM") as pattn,
        ):
            # ---- constants (DMA order matters: vproj prereqs first) ----
            ones_sb = cpool.tile([1, 128], BF16, tag="ones")
            nc.sync.dma_start(ones_sb[:], ones_d[:])
            bv_sb = cpool.tile([1, HL * DH], BF16, tag="bv")
            nc.sync.dma_start(bv_sb[:], bv_d[:])

            # xT first-column chunks and wv interleaved across three queues,
            # ordered by the first vproj chain's consumption (kb order): with
            # ~5us fixed DMA latency, queue POSITION sets arrival order.
            xT_sb = []
            for kb in range(KB):
                t = xpool.tile([128, N], BF16, tag=f"xt{kb}", name=f"xt{kb}")
                xT_sb.append(t)
            wv_sb = [
                cpool.tile([128, HL * DH], BF16, tag=f"wv{kb}", name=f"wv{kb}")
                for kb in range(KB)
            ]

            def wv_dma(eng, kb):
                eng.dma_start(wv_sb[kb][:], wv_d[kb * 128:(kb + 1) * 128, :])

            def xt0_dma(eng, kb):
                eng.dma_start(
                    xT_sb[kb][:, 0:512], xT_d[kb * 128:(kb + 1) * 128, 0:512]
                )

            wv_dma(nc.scalar, 0)
            xt0_dma(nc.gpsimd, 0)
            wv_dma(nc.scalar, 1)
            xt0_dma(nc.gpsimd, 1)
            xt0_dma(nc.sync, 2)
            xt0_dma(nc.sync, 3)
            wv_dma(nc.gpsimd, 5)
            xt0_dma(nc.scalar, 4)
            xt0_dma(nc.scalar, 5)
            wv_dma(nc.sync, 3)
            wv_dma(nc.scalar, 2)
            wv_dma(nc.sync, 4)

            def xt_dma(eng, kb, c):
                eng.dma_start(
                    xT_sb[kb][:, c * 512:(c + 1) * 512],
                    xT_d[kb * 128:(kb + 1) * 128, c * 512:(c + 1) * 512],
                )

            # load the exp ACT table set off the critical path
            scr = npool.tile([1, 16], F32, tag="scr", bufs=1)
            nc.scalar.activation(scr[:], ones_sb[:, 0:16], AF.Exp)
            bq_sb = cpool.tile([D, HL], F32, tag="bq")
            nc.sync.dma_start(bq_sb[:], bq_d[:])
            bk_sb = cpool.tile([D, HL], F32, tag="bk")
            nc.sync.dma_start(bk_sb[:], bk_d[:])
            bp_sb = cpool.tile([1, E], BF16, tag="bp")
            nc.sync.dma_start(bp_sb[:], bp_d[:])

            # wq on sync (q-chunks run first and need only xT c0 + wq); wk on
            # gpsimd AFTER xT c1 so its 0.59MB doesn't compete for HBM during
            # the bandwidth-bound first ~8us (k0 isn't consumed until ~+16us)
            wq_sb = []
            wk_sb = []
            for kb in range(KB):
                t = cpool.tile([128, EL], BF16, tag=f"wq{kb}")
                nc.sync.dma_start(t[:], wq_d[kb * 128:(kb + 1) * 128, :])
                wq_sb.append(t)
            for kb in range(KB):
                xt_dma(nc.gpsimd, kb, 1)
            for kb in range(KB):
                t = cpool.tile([128, EL], BF16, tag=f"wk{kb}")
                nc.gpsimd.dma_start(t[:], wk_d[kb * 128:(kb + 1) * 128, :])
                wk_sb.append(t)
            for c in (2, 3):
                for kb in range(KB):
                    xt_dma(nc.gpsimd, kb, c)
            # wp loads are deferred into the head-1 attention pass: they are
            # not needed until the output projection (~80% through) and would
            # steal startup HBM bandwidth from xT/wq/wk.
            wp_sb = []

            def load_wp():
                for g in range(3):
                    t = cpool.tile([128, E], BF16, tag=f"wp{g}")
                    nc.gpsimd.dma_start(t[:], wp_d[g * 128:(g + 1) * 128, :])
                    wp_sb.append(t)

            # broadcast bias tiles (one K=1 matmul each, reused everywhere)
            bvb_sb = cpool.tile([128, HL * DH], BF16, tag="bvb")
            ps = pp.tile([128, 512], F32, tag="pp")
            nc.tensor.matmul(ps[:, 0:HL * DH], ones_sb[:], bv_sb[:], start=True, stop=True)
            nc.vector.tensor_copy(bvb_sb[:], ps[:, 0:HL * DH])
            bpb_sb = cpool.tile([128, E], BF16, tag="bpb")

            # normalized per-head outputs, heads stacked along partitions:
            # otn[qh][g][128g + r, i] = OT_(r//96)[r % 96, qh*1024 + i]
            otn = [
                [
                    opool.tile(
                        [128, 1024], BF16, tag=f"otn{qh}_{g}", name=f"otn{qh}_{g}"
                    )
                    for g in range(3)
                ]
                for qh in range(2)
            ]

            def start_qkproj(h):
                # q is kept as an fp8 (hi, lo) residual pair so the DoubleRow
                # scores matmul sees q at ~bf16 precision; k is quantized once
                # to fp8 (its pair dim is a stride-0 broadcast in the matmul).
                with nc.named_scope(f"qkproj{h}"):
                    nq = 1 if SCORE_MODE != "dr_resid" else 2
                    qT = qkpool.tile([D, nq, N], QKDT, tag="qT", name=f"qT{h}")
                    kT = qkpool.tile([D, N], QKDT, tag="kT", name=f"kT{h}")
                return (qT, kT)

            def emit_qkproj_chunk(h, tiles, i):
                qT, kT = tiles
                qk, c = divmod(i, 4)
                w_sb, b_sb = [(wq_sb, bq_sb), (wk_sb, bk_sb)][qk]
                sl = slice(c * 512, (c + 1) * 512)
                with nc.named_scope(f"qkproj{h}"):
                    ps = pp.tile([128, 512], F32, tag="pp", name=f"psqk{h}_{i}")
                    for kb in range(KB):
                        nc.tensor.matmul(
                            ps[0:D, :],
                            w_sb[kb][:, h * D:(h + 1) * D],
                            xT_sb[kb][:, sl],
                            start=(kb == 0),
                            stop=(kb == KB - 1),
                        )
                    if qk == 1:
                        nc.vector.tensor_scalar(
                            kT[:, sl], ps[0:D, :], 1.0, b_sb[:, h:h + 1], MULT, ADD
                        )
                    elif SCORE_MODE == "dr_resid":
                        qtmp = npool.tile([D, 512], BF16, tag="qtmp", bufs=2)
                        nc.vector.tensor_scalar(
                            qtmp[:], ps[0:D, :], 1.0, b_sb[:, h:h + 1], MULT, ADD
                        )
                        nc.vector.tensor_copy(qT[:, 0, sl], qtmp[:])
                        nc.vector.tensor_tensor(
                            qT[:, 1, sl], qtmp[:], qT[:, 0, sl], SUB
                        )
                    else:
                        nc.vector.tensor_scalar(
                            qT[:, 0, sl], ps[0:D, :], 1.0, b_sb[:, h:h + 1],
                            MULT, ADD,
                        )

            # ---- V-hat projection interleaved with head-0 qk projection ----
            # Only vproj c0-c2, q0, q1 and k0 run up front: the startup region
            # is HBM-bandwidth-bound, so the rest of head-0's chunks and the
            # vproj c3 block group are deferred into the attn0_0 pass.
            vhat = []

            def emit_vproj_block(nb):
                with nc.named_scope("vproj"):
                    ps = pp.tile([128, 512], F32, tag="pp")
                    for kb in range(KB):
                        nc.tensor.matmul(
                            ps[:, 0:HL * DH],
                            xT_sb[kb][:, nb * 128:(nb + 1) * 128],
                            wv_sb[kb][:],
                            start=(kb == 0),
                            stop=(kb == KB - 1),
                        )
                    vt = vpool.tile([128, HL * DH], BF16, tag=f"vh{nb}")
                    nc.vector.tensor_tensor(vt[:], ps[:, 0:HL * DH], bvb_sb[:], ADD)
                    vhat.append(vt)

            tiles = start_qkproj(0)
            for c in range(4):
                for nb in range(4 * c, 4 * c + 4):
                    emit_vproj_block(nb)
                # q0, q1, k0, k1 up front; q2/q3/k2/k3 defer into attn0_0
                emit_qkproj_chunk(0, tiles, [0, 1, 4, 5][c])
                if c == 1:
                    # build the bproj broadcast (off the critical start) in a
                    # free attention-et PSUM slot (one per bank half) so it
                    # doesn't contend with the qkproj chains' pp slots
                    ps = pattn.tile([128, 1024], F32, tag="et", bufs=2)
                    for off, w in [(0, 512), (512, 256)]:
                        nc.tensor.matmul(
                            ps[:, off:off + w], ones_sb[:], bp_sb[:, off:off + w],
                            start=True, stop=True,
                        )
                    nc.vector.tensor_copy(bpb_sb[:], ps[:, 0:E])

            def drain_norm(h, qh, acc, tail=False):
                """Drain the PSUM accumulator (bf16 rows + f32 denominator
                row) so the next pass's PV(0) gets the buffer back fast, and
                run the reciprocal chains (DVE only — ACT still has an exp
                backlog at the pass boundary, so any ACT op here delays the
                next pass's exps and starves the PE of et buffers). Tail:
                per-j chains with ACT offload so the j=0 half unblocks the
                output projection early."""
                with nc.named_scope(f"norm{h}_{qh}"):
                    if not tail:
                        a = npool.tile([D, 1024], BF16, tag="acc_bf")
                        nc.vector.tensor_copy(a[:], acc[0:D, :])
                        sums = npool.tile([1, 1024], F32, tag="sums")
                        nc.vector.tensor_copy(sums[:], acc[D:DH, :])
                        rec = npool.tile([1, 1024], F32, tag="rec")
                        nc.vector.reciprocal_approx_fast(rec[:], sums[:])
                        recb = npool.tile([1, 1024], BF16, tag="recb")
                        nc.vector.tensor_copy(recb[:], rec[:])
                        accbf = [a[:, 0:512], a[:, 512:1024]]
                        recbs = [recb[:, 0:512], recb[:, 512:1024]]
                        return (h, qh, accbf, recbs)
                    accbf, recbs = [], []
                    for j in range(2):
                        sl = slice(j * 512, (j + 1) * 512)
                        sums = npool.tile([1, 512], F32, tag="sumst")
                        if j == 0:
                            nc.vector.tensor_copy(sums[:], acc[D:DH, sl])
                        else:
                            nc.scalar.copy(sums[:], acc[D:DH, sl])
                        rec = npool.tile([1, 512], F32, tag="rect")
                        nc.vector.reciprocal_approx_fast(rec[:], sums[:])
                        recb = npool.tile([1, 512], BF16, tag="recbt")
                        if j == 0:
                            nc.vector.tensor_copy(recb[:], rec[:])
                        else:
                            nc.scalar.copy(recb[:], rec[:])
                        a = npool.tile([D, 512], BF16, tag="acc_bft")
                        nc.scalar.copy(a[:], acc[0:D, sl])
                        accbf.append(a[:])
                        recbs.append(recb[:])
                return (h, qh, accbf, recbs)

            def finish_norm(h, qh, accbf, recbs):
                """Scale by the reciprocal row (read partition-broadcast, so
                no PE ones-matmul is needed) and stack into the K=128-packed
                otn layout. Emitted a few kb into the following pass."""
                with nc.named_scope(f"norm{h}_{qh}"):
                    for j in range(2):
                        sl = slice(j * 512, (j + 1) * 512)
                        # (gpsimd partition_broadcast here measured WORSE:
                        # its queue latency in this chain cascades into pass-
                        # boundary stalls; the PE ones-matmul stays)
                        bc = pp.tile([128, 512], F32, tag="pp")
                        nc.tensor.matmul(
                            bc[0:D, :], ones_sb[:, 0:D], recbs[j],
                            start=True, stop=True,
                        )
                        rbb = bc[0:D, :]
                        if h == 0:
                            # rows 0..95 land on the same partitions: write the
                            # stack tile directly, no shift DMA needed
                            nc.vector.tensor_tensor(
                                otn[qh][0][0:D, sl], accbf[j], rbb, MULT
                            )
                            continue
                        ot = npool.tile([D, 512], BF16, tag="ot", bufs=4)
                        nc.vector.tensor_tensor(ot[:], accbf[j], rbb, MULT)
                        # stack rows 96h..96h+95 into the K=128-packed layout
                        # (DMAs alternate queues so the tail stacks land fast)
                        r0 = D * h
                        g0, off = divmod(r0, 128)
                        n0 = min(128 - off, D)
                        e1, e2 = (nc.sync, nc.scalar) if j == 0 else (nc.scalar, nc.sync)
                        e1.dma_start(otn[qh][g0][off:off + n0, sl], ot[0:n0, :])
                        if n0 < D:
                            e2.dma_start(
                                otn[qh][g0 + 1][0:D - n0, sl], ot[n0:D, :]
                            )

            def emit_out(nb, slot):
                qh, col = divmod(nb, 8)
                col *= 128
                with nc.named_scope(f"out{nb}"):
                    AB = None
                    if slot == "pp":
                        pa = pp.tile([128, 512], F32, tag="pp")
                        pb = pp.tile([128, 512], F32, tag="pp")
                        A, Bv = pa[:, 0:512], pb[:, 0:256]
                    elif slot == "acc":
                        t = pattn.tile([128, 1024], F32, tag="acc", bufs=1)
                        A, Bv, AB = t[:, 0:512], t[:, 512:768], t[:, 0:768]
                    else:
                        t = pattn.tile([128, 1024], F32, tag="et", bufs=2)
                        A, Bv, AB = t[:, 0:512], t[:, 512:768], t[:, 0:768]
                    for g in range(3):
                        nc.tensor.matmul(
                            A,
                            otn[qh][g][:, col:col + 128],
                            wp_sb[g][:, 0:512],
                            start=(g == 0),
                            stop=(g == 2),
                        )
                    for g in range(3):
                        nc.tensor.matmul(
                            Bv,
                            otn[qh][g][:, col:col + 128],
                            wp_sb[g][:, 512:768],
                            start=(g == 0),
                            stop=(g == 2),
                        )
                    osb = npool.tile([128, E], BF16, tag="osb", bufs=4)
                    if AB is not None:
                        # both psum halves live in one tile: one 768-wide add
                        nc.vector.tensor_tensor(osb[:], AB, bpb_sb[:], ADD)
                    else:
                        nc.vector.tensor_tensor(osb[:, 0:512], A, bpb_sb[:, 0:512], ADD)
                        nc.vector.tensor_tensor(
                            osb[:, 512:768], Bv, bpb_sb[:, 512:768], ADD
                        )
                    # spread across three queues so the tail transfers drain
                    # in parallel (exec time runs to the last DMA completion)
                    eng = [nc.sync, nc.scalar, nc.gpsimd][nb % 3]
                    eng.dma_start(out_d[nb * 128:(nb + 1) * 128, :], osb[:])

            # ---- per-head attention; qkproj of h+1 spread over both passes ----
            pending_norm = None
            for h in range(HL):
                qT, kT = tiles
                nxt = None
                for qh in range(2):
                    with nc.named_scope(f"attn{h}_{qh}"):
                        acc = pattn.tile([128, 1024], F32, tag="acc")

                        def emit_pv(kbp, pt):
                            for j in range(2):
                                nc.tensor.matmul(
                                    acc[0:DH, j * 512:(j + 1) * 512],
                                    vhat[kbp][:, h * DH:(h + 1) * DH],
                                    pt[:, j * 512:(j + 1) * 512],
                                    start=(kbp == 0),
                                    stop=(kbp == NB - 1),
                                )

                        pts = []
                        for kb in range(NB):
                            # exp stays one [128, 1024] op: splitting it into
                            # halves costs ~80ns/op of ACT overhead (+19us of
                            # exp time overall — measured regression)
                            et = pattn.tile([128, 1024], F32, tag="et", bufs=2)
                            if SCORE_MODE == "dr_resid":
                                kTb = kT[:, kb * 128:(kb + 1) * 128].unsqueeze(
                                    1
                                ).broadcast_to((D, 2, 128))
                            else:
                                kTb = kT[:, kb * 128:(kb + 1) * 128]
                            for j in range(2):
                                c = 2 * qh + j
                                if SCORE_MODE == "dr_resid":
                                    rhs = qT[:, :, c * 512:(c + 1) * 512]
                                else:
                                    rhs = qT[:, 0, c * 512:(c + 1) * 512]
                                nc.tensor.matmul(
                                    et[:, j * 512:(j + 1) * 512],
                                    kTb,
                                    rhs,
                                    start=True,
                                    stop=True,
                                    perf_mode=SCORE_PERF,
                                )
                            if kb >= LAG:
                                emit_pv(kb - LAG, pts[kb - LAG])
                            pt = ptpool.tile([128, 1024], BF16, tag="pt")
                            nc.scalar.activation(pt[:], et[:], AF.Exp, scale=SCALE)
                            pts.append(pt)
                            fin_kb = 3 if (h == 3 and qh == 1) else LAG
                            if kb == fin_kb and pending_norm is not None:
                                # previous pass's normalization: by now its
                                # reciprocal chain is done, so the broadcast
                                # matmuls won't stall the PE queue. One step
                                # earlier in the last pass so the otn stacks
                                # land before the first interleaved out block.
                                finish_norm(*pending_norm)
                                pending_norm = None
                            if h == 1 and qh == 0 and kb == 6:
                                load_wp()
                            g = qh * 16 + kb
                            if h + 1 < HL and g % 4 == 0:
                                # h3's k1..k3 chunks are deferred into its own
                                # first pass (which has no next-head projection
                                # and would idle); the remaining five h3 chunks
                                # spread 3+2 over h2's PE-bound passes.
                                if g == 0:
                                    nxt = start_qkproj(h + 1)
                                if h == 2:
                                    idx = {0: 0, 4: 1, 8: 2, 16: 3, 20: 4}.get(g)
                                    if idx is not None:
                                        emit_qkproj_chunk(3, nxt, idx)
                                else:
                                    emit_qkproj_chunk(h + 1, nxt, g // 4)
                            if h == 3 and qh == 0 and kb in (0, 1, 2):
                                # bunched at kb0-2: covers the PE idle window
                                # while ACT drains the previous pass's exps
                                emit_qkproj_chunk(3, (qT, kT), 5 + kb)
                            if h == 0 and qh == 0 and kb in (1, 5, 9, 11):
                                # head-0 work deferred out of the bw-bound
                                # startup: k2/k3 ready well before their
                                # scores (kb 8/12), q2/q3 before attn0_1
                                emit_qkproj_chunk(
                                    0, (qT, kT), {1: 6, 5: 7, 9: 2, 11: 3}[kb]
                                )
                            if h == 3 and qh == 1 and kb >= 5 and kb % 2 == 1:
                                emit_out((kb - 5) // 2, "pp")
                        for kbp in range(NB - LAG, NB):
                            emit_pv(kbp, pts[kbp])
                        pending_norm = drain_norm(h, qh, acc, tail=(h == HL - 1 and qh == 1))
                tiles = nxt

            # ---- output projection tail ----
            # nb 6/7 only need the qh=0 stacks: they keep the PE busy while
            # the last norm's reciprocal chain runs
            emit_out(6, "pp")
            finish_norm(*pending_norm)
            emit_out(7, "et")
            # 4-deep PSUM slot rotation (et alternates its two buffers) so
            # slot recycling isn't gated on the tail's busy DVE
            for nb, slot in zip(range(8, NB), ("acc", "et", "pp", "et", "acc", "et", "pp", "et")):
                emit_out(nb, slot)

    nc.compile()
    return nc


def _shard(x, Wqkv, bqkv, Wproj, bproj):
    """Build per-core input maps. Core c -> (batch c//2, head-group c%2)."""
    import ml_dtypes

    bf16 = ml_dtypes.bfloat16
    Wr = np.ascontiguousarray(Wqkv.reshape(E, H, D, 3))
    br = np.ascontiguousarray(bqkv.reshape(H, D, 3))
    ones = np.ones((1, 128), bf16)
    in_maps = []
    for c in range(NC):
        bb, hg = divmod(c, 2)
        hs = slice(hg * HL, (hg + 1) * HL)
        wq = np.ascontiguousarray(Wr[:, hs, :, 0].reshape(E, EL)).astype(bf16)
        wk = np.ascontiguousarray(Wr[:, hs, :, 1].reshape(E, EL)).astype(bf16)
        wv = np.zeros((E, HL, DH), np.float32)
        wv[:, :, :D] = Wr[:, hs, :, 2]
        bq = np.ascontiguousarray(br[hs, :, 0].T)  # [D, HL] (scale applied at exp)
        bk = np.ascontiguousarray(br[hs, :, 1].T)
        bv = np.zeros((HL, DH), np.float32)
        bv[:, :D] = br[hs, :, 2]
        bv[:, D] = 1.0  # denominator ones column
        wp = np.ascontiguousarray(Wproj[hg * EL:(hg + 1) * EL, :]).astype(bf16)
        bp = bproj if hg == 0 else np.zeros_like(bproj)
        in_maps.append({
            "xT": np.ascontiguousarray(x[bb].T).astype(bf16),
            "wq": wq,
            "wk": wk,
            "wv": np.ascontiguousarray(wv.reshape(E, HL * DH)).astype(bf16),
            "bq": np.ascontiguousarray(bq, dtype=np.float32),
            "bk": np.ascontiguousarray(bk, dtype=np.float32),
            "bv": np.ascontiguousarray(bv.reshape(1, HL * DH)).astype(bf16),
            "wp": wp,
            "bp": np.ascontiguousarray(bp.reshape(1, E)).astype(bf16),
            "ones": ones,
            "ones32": np.ones((1, 128), np.float32),
        })
    return in_maps


def kernel(x, Wqkv, bqkv, Wproj, bproj):
    global _COMPILED, LAST_EXEC_NS, LAST_RESULTS
    x = np.asarray(x, dtype=np.float32)
    Wqkv = np.asarray(Wqkv, dtype=np.float32)
    bqkv = np.asarray(bqkv, dtype=np.float32)
    Wproj = np.asarray(Wproj, dtype=np.float32)
    bproj = np.asarray(bproj, dtype=np.float32)

    if _COMPILED is None:
        _COMPILED = _build()
    nc = _COMPILED

    in_maps = _shard(x, Wqkv, bqkv, Wproj, bproj)
    trace = bool(int(os.environ.get("BASS_MHA_TRACE", "0")))
    try:
        res = run_bass_kernel_spmd(nc, in_maps, list(range(NC)), trace=trace)
    except Exception:
        _device_reset()
        res = run_bass_kernel_spmd(nc, in_maps, list(range(NC)), trace=trace)
    LAST_EXEC_NS = res.exec_time_ns
    LAST_RESULTS = res

    out = np.empty((B, N, E), np.float32)
    for bb in range(B):
        out[bb] = res.results[2 * bb]["out"].astype(np.float32) + res.results[
            2 * bb + 1
        ]["out"].astype(np.float32)
    return out



# revision 78
# speedup vs baseline: 1.2910x; 1.0065x over previous
"""Multi-head attention TRN2 kernel (b=4, n=2048, e=768, h=8 heads, d=96).

Sharding: 8 cores = 4 batches x 2 head-groups (4 heads each).
Each core computes, for its (batch, head-group):
    qkv projection (its heads' columns of Wqkv), per-head attention
    (softmax over full n=2048), and a partial output projection
    (its heads' rows of Wproj). Host sums the two bf16 partial outputs
    per batch in f32 (row-parallel linear unshard) and concatenates.

All matmul operands are bf16 (fp32 PSUM accumulation): the TRN2 PE
streams exactly one moving column per cycle for every dtype (fp8
DoubleRow/DoubleColumn only extend the contraction depth, measured on
hw), so bf16 is already at peak column rate and fp8 would only add
quantization error. Scores are computed transposed (ET[nk, nq]) so no
transposes are needed; 1/sqrt(e) is folded into the exp activation's
scale; softmax denominators come from an extra ones-column appended to V
(row 96 of the PV accumulator). exp() skips max-subtraction: scaled
logits are bounded (~|2|). Per-head normalized outputs stay in SBUF and
are restacked to a K=128-packed [3 x 128, n] layout with SBUF->SBUF
DMAs; the output projection then runs 3 matmuls per chunk.

Schedule notes (all measured on hw):
- PV matmuls run LAG kb-steps behind exp; the LAG-deep PV tail covers
  the ACT exp backlog at pass boundaries. The norm drain uses DVE only
  (an ACT op there delays the next pass's exps and starves the PE of
  et buffers).
- The startup is HBM-bandwidth/latency-bound: only vproj c0-c2 and
  q0/q1/k0/k1 of head 0 run up front; head-0's q2/q3/k2/k3 defer into
  attn0_0. wp loads defer to mid-kernel; wk rides gpsimd behind xT c1.
- h3's k1-k3 chunks are emitted in its own first pass (bunched at kb
  0-2 to cover the boundary backlog); the remaining h3 chunks spread
  3+2 over h2's passes.
- out blocks 0-5 interleave into the last pass (stacks ready early via
  finish_norm at kb3); the tail rotates out8-15 over four PSUM slots.
"""

import os

import numpy as np

import concourse.bacc as bacc
import concourse.mybir as mybir
import concourse.tile as tile
from concourse.bass_utils import run_bass_kernel_spmd

B, N, E = 4, 2048, 768
H = 8          # total heads
HL = 4         # heads per core
D = E // H     # 96
DH = D + 1     # 97 (with denominator column)
KB = E // 128  # 6 contraction blocks
NB = N // 128  # 16 row blocks
NC = 8         # cores
EL = HL * D    # 384 local e-dim
SCALE = float(E) ** -0.5
LAG = 5        # PV runs LAG kb-steps behind exp

F32 = mybir.dt.float32
BF16 = mybir.dt.bfloat16
F8 = mybir.dt.float8e4
DR = mybir.MatmulPerfMode.DoubleRow
# 'bf16': plain bf16 scores (best error, PE col rate is 1/cycle regardless).
# 'dr_resid': fp8 DoubleRow, q as (hi, lo) residual pair — same speed as bf16.
# 'dc_plain': fp8 DoubleColumn, q/k plain fp8 — measured same speed as bf16.
SCORE_MODE = os.environ.get("SCORE_MODE", "bf16")
SCORE_PERF = {
    "bf16": None,
    "dr_resid": DR,
    "dc_plain": mybir.MatmulPerfMode.DoubleColumn,
}[SCORE_MODE]
QKDT = BF16 if SCORE_MODE == "bf16" else F8
AF = mybir.ActivationFunctionType
MULT = mybir.AluOpType.mult
ADD = mybir.AluOpType.add
SUB = mybir.AluOpType.subtract

_COMPILED = None
LAST_EXEC_NS = None
LAST_RESULTS = None


def _device_reset():
    """Recover a wedged NeuronCore (NRT_EXEC_UNIT_UNRECOVERABLE) via axon."""
    try:
        import ctypes
        import time

        import jax

        jax.devices()
        lib = ctypes.CDLL("/opt/axon/libaxon_pjrt.so")
        lib.axon_reset.restype = ctypes.c_int64
        lib.axon_reset()
        time.sleep(3)
    except Exception:
        pass


def _build():
    nc = bacc.Bacc("TRN2", target_bir_lowering=False, debug=False)

    wvxt_d = nc.dram_tensor("wvxt", [E, HL * DH + N], BF16, kind="ExternalInput")
    wq_d = nc.dram_tensor("wq", [E, EL], BF16, kind="ExternalInput")
    wk_d = nc.dram_tensor("wk", [E, EL], BF16, kind="ExternalInput")
    bq_d = nc.dram_tensor("bq", [D, HL], F32, kind="ExternalInput")
    bk_d = nc.dram_tensor("bk", [D, HL], F32, kind="ExternalInput")
    bv_d = nc.dram_tensor("bv", [1, HL * DH], BF16, kind="ExternalInput")
    wp_d = nc.dram_tensor("wp", [EL, E], BF16, kind="ExternalInput")
    bp_d = nc.dram_tensor("bp", [1, E], BF16, kind="ExternalInput")
    ones_d = nc.dram_tensor("ones", [1, 128], BF16, kind="ExternalInput")
    ones32_d = nc.dram_tensor("ones32", [1, 128], F32, kind="ExternalInput")
    out_d = nc.dram_tensor("out", [N, E], BF16, kind="ExternalOutput")

    with tile.TileContext(nc) as tc:
        with (
            tc.tile_pool(name="const", bufs=1) as cpool,
            tc.tile_pool(name="xt", bufs=1) as xpool,
            tc.tile_pool(name="qk", bufs=2) as qkpool,
            tc.tile_pool(name="vh", bufs=1) as vpool,
            tc.tile_pool(name="pt", bufs=LAG + 3) as ptpool,
            tc.tile_pool(name="nrm", bufs=3) as npool,
            tc.tile_pool(name="on", bufs=1) as opool,
            tc.tile_pool(name="pp", bufs=2, space="PSUM") as pp,
            tc.tile_pool(name="pattn", bufs=1, space="PSUM") as pattn,
        ):
            # ---- constants (DMA order matters: vproj prereqs first) ----
            ones_sb = cpool.tile([1, 128], BF16, tag="ones")
            nc.sync.dma_start(ones_sb[:], ones_d[:])
            bv_sb = cpool.tile([1, HL * DH], BF16, tag="bv")
            nc.sync.dma_start(bv_sb[:], bv_d[:])

            # Each kb row-block's wv and xT live in ONE packed tile
            # ("wvxt", wv cols 0:388 then xT cols 388:388+N): the first vproj
            # chain then needs just SIX transfers (two per queue) instead of
            # twelve, and per-queue completion spacing gates its start.
            WX = HL * DH
            xT_sb = []
            for kb in range(KB):
                t = xpool.tile([128, WX + N], BF16, tag=f"xt{kb}", name=f"xt{kb}")
                xT_sb.append(t)
            wv_sb = xT_sb  # wv operand is cols 0:WX of the packed tile

            for kb, eng in zip(
                range(KB),
                [nc.gpsimd, nc.sync, nc.scalar, nc.gpsimd, nc.sync, nc.scalar],
            ):
                eng.dma_start(
                    xT_sb[kb][:, 0:WX + 512],
                    wvxt_d[kb * 128:(kb + 1) * 128, 0:WX + 512],
                )

            def xt_dma(eng, kb, c):
                eng.dma_start(
                    xT_sb[kb][:, WX + c * 512:WX + (c + 1) * 512],
                    wvxt_d[kb * 128:(kb + 1) * 128, WX + c * 512:WX + (c + 1) * 512],
                )

            # load the exp ACT table set off the critical path
            scr = npool.tile([1, 16], F32, tag="scr", bufs=1)
            nc.scalar.activation(scr[:], ones_sb[:, 0:16], AF.Exp)
            bq_sb = cpool.tile([D, HL], F32, tag="bq")
            nc.sync.dma_start(bq_sb[:], bq_d[:])
            bk_sb = cpool.tile([D, HL], F32, tag="bk")
            nc.sync.dma_start(bk_sb[:], bk_d[:])
            bp_sb = cpool.tile([1, E], BF16, tag="bp")
            nc.sync.dma_start(bp_sb[:], bp_d[:])

            # wq on sync (q-chunks run first and need only xT c0 + wq); wk on
            # gpsimd AFTER xT c1 so its 0.59MB doesn't compete for HBM during
            # the bandwidth-bound first ~8us (k0 isn't consumed until ~+16us)
            wq_sb = []
            wk_sb = []
            for kb in range(KB):
                t = cpool.tile([128, EL], BF16, tag=f"wq{kb}")
                nc.sync.dma_start(t[:], wq_d[kb * 128:(kb + 1) * 128, :])
                wq_sb.append(t)
            for kb in range(KB):
                xt_dma(nc.gpsimd, kb, 1)
            for kb in range(KB):
                t = cpool.tile([128, EL], BF16, tag=f"wk{kb}")
                nc.gpsimd.dma_start(t[:], wk_d[kb * 128:(kb + 1) * 128, :])
                wk_sb.append(t)
            for c in (2, 3):
                for kb in range(KB):
                    xt_dma(nc.gpsimd, kb, c)
            # wp loads are deferred into the head-1 attention pass: they are
            # not needed until the output projection (~80% through) and would
            # steal startup HBM bandwidth from xT/wq/wk.
            wp_sb = []

            def load_wp():
                for g in range(3):
                    t = cpool.tile([128, E], BF16, tag=f"wp{g}")
                    nc.gpsimd.dma_start(t[:], wp_d[g * 128:(g + 1) * 128, :])
                    wp_sb.append(t)

            # broadcast bias tiles (one K=1 matmul each, reused everywhere)
            bvb_sb = cpool.tile([128, HL * DH], BF16, tag="bvb")
            ps = pp.tile([128, 512], F32, tag="pp")
            nc.tensor.matmul(ps[:, 0:HL * DH], ones_sb[:], bv_sb[:], start=True, stop=True)
            nc.vector.tensor_copy(bvb_sb[:], ps[:, 0:HL * DH])
            bpb_sb = cpool.tile([128, E], BF16, tag="bpb")

            # normalized per-head outputs, heads stacked along partitions:
            # otn[qh][g][128g + r, i] = OT_(r//96)[r % 96, qh*1024 + i]
            otn = [
                [
                    opool.tile(
                        [128, 1024], BF16, tag=f"otn{qh}_{g}", name=f"otn{qh}_{g}"
                    )
                    for g in range(3)
                ]
                for qh in range(2)
            ]

            def start_qkproj(h):
                # q is kept as an fp8 (hi, lo) residual pair so the DoubleRow
                # scores matmul sees q at ~bf16 precision; k is quantized once
                # to fp8 (its pair dim is a stride-0 broadcast in the matmul).
                with nc.named_scope(f"qkproj{h}"):
                    nq = 1 if SCORE_MODE != "dr_resid" else 2
                    qT = qkpool.tile([D, nq, N], QKDT, tag="qT", name=f"qT{h}")
                    kT = qkpool.tile([D, N], QKDT, tag="kT", name=f"kT{h}")
                return (qT, kT)

            def emit_qkproj_chunk(h, tiles, i):
                qT, kT = tiles
                qk, c = divmod(i, 4)
                w_sb, b_sb = [(wq_sb, bq_sb), (wk_sb, bk_sb)][qk]
                sl = slice(c * 512, (c + 1) * 512)
                xsl = slice(WX + c * 512, WX + (c + 1) * 512)
                with nc.named_scope(f"qkproj{h}"):
                    ps = pp.tile([128, 512], F32, tag="pp", name=f"psqk{h}_{i}")
                    for kb in range(KB):
                        nc.tensor.matmul(
                            ps[0:D, :],
                            w_sb[kb][:, h * D:(h + 1) * D],
                            xT_sb[kb][:, xsl],
                            start=(kb == 0),
                            stop=(kb == KB - 1),
                        )
                    if qk == 1:
                        nc.vector.tensor_scalar(
                            kT[:, sl], ps[0:D, :], 1.0, b_sb[:, h:h + 1], MULT, ADD
                        )
                    elif SCORE_MODE == "dr_resid":
                        qtmp = npool.tile([D, 512], BF16, tag="qtmp", bufs=2)
                        nc.vector.tensor_scalar(
                            qtmp[:], ps[0:D, :], 1.0, b_sb[:, h:h + 1], MULT, ADD
                        )
                        nc.vector.tensor_copy(qT[:, 0, sl], qtmp[:])
                        nc.vector.tensor_tensor(
                            qT[:, 1, sl], qtmp[:], qT[:, 0, sl], SUB
                        )
                    else:
                        nc.vector.tensor_scalar(
                            qT[:, 0, sl], ps[0:D, :], 1.0, b_sb[:, h:h + 1],
                            MULT, ADD,
                        )

            # ---- V-hat projection interleaved with head-0 qk projection ----
            # Only vproj c0-c2, q0, q1 and k0 run up front: the startup region
            # is HBM-bandwidth-bound, so the rest of head-0's chunks and the
            # vproj c3 block group are deferred into the attn0_0 pass.
            vhat = []

            def emit_vproj_block(nb):
                with nc.named_scope("vproj"):
                    ps = pp.tile([128, 512], F32, tag="pp")
                    for kb in range(KB):
                        nc.tensor.matmul(
                            ps[:, 0:HL * DH],
                            xT_sb[kb][:, WX + nb * 128:WX + (nb + 1) * 128],
                            wv_sb[kb][:, 0:WX],
                            start=(kb == 0),
                            stop=(kb == KB - 1),
                        )
                    vt = vpool.tile([128, HL * DH], BF16, tag=f"vh{nb}")
                    nc.vector.tensor_tensor(vt[:], ps[:, 0:HL * DH], bvb_sb[:], ADD)
                    vhat.append(vt)

            tiles = start_qkproj(0)
            for c in range(4):
                for nb in range(4 * c, 4 * c + 4):
                    emit_vproj_block(nb)
                # q0, q1, k0, k1 up front; q2/q3/k2/k3 defer into attn0_0
                emit_qkproj_chunk(0, tiles, [0, 1, 4, 5][c])
                if c == 1:
                    # build the bproj broadcast (off the critical start) in a
                    # free attention-et PSUM slot (one per bank half) so it
                    # doesn't contend with the qkproj chains' pp slots
                    ps = pattn.tile([128, 1024], F32, tag="et", bufs=2)
                    for off, w in [(0, 512), (512, 256)]:
                        nc.tensor.matmul(
                            ps[:, off:off + w], ones_sb[:], bp_sb[:, off:off + w],
                            start=True, stop=True,
                        )
                    nc.vector.tensor_copy(bpb_sb[:], ps[:, 0:E])

            def drain_norm(h, qh, acc, tail=False):
                """Drain the PSUM accumulator (bf16 rows + f32 denominator
                row) so the next pass's PV(0) gets the buffer back fast, and
                run the reciprocal chains (DVE only — ACT still has an exp
                backlog at the pass boundary, so any ACT op here delays the
                next pass's exps and starves the PE of et buffers). Tail:
                per-j chains with ACT offload so the j=0 half unblocks the
                output projection early."""
                with nc.named_scope(f"norm{h}_{qh}"):
                    if not tail:
                        a = npool.tile([D, 1024], BF16, tag="acc_bf")
                        nc.vector.tensor_copy(a[:], acc[0:D, :])
                        sums = npool.tile([1, 1024], F32, tag="sums")
                        nc.vector.tensor_copy(sums[:], acc[D:DH, :])
                        rec = npool.tile([1, 1024], F32, tag="rec")
                        nc.vector.reciprocal_approx_fast(rec[:], sums[:])
                        recb = npool.tile([1, 1024], BF16, tag="recb")
                        nc.vector.tensor_copy(recb[:], rec[:])
                        accbf = [a[:, 0:512], a[:, 512:1024]]
                        recbs = [recb[:, 0:512], recb[:, 512:1024]]
                        return (h, qh, accbf, recbs)
                    accbf, recbs = [], []
                    for j in range(2):
                        sl = slice(j * 512, (j + 1) * 512)
                        sums = npool.tile([1, 512], F32, tag="sumst")
                        if j == 0:
                            nc.vector.tensor_copy(sums[:], acc[D:DH, sl])
                        else:
                            nc.scalar.copy(sums[:], acc[D:DH, sl])
                        rec = npool.tile([1, 512], F32, tag="rect")
                        nc.vector.reciprocal_approx_fast(rec[:], sums[:])
                        recb = npool.tile([1, 512], BF16, tag="recbt")
                        if j == 0:
                            nc.vector.tensor_copy(recb[:], rec[:])
                        else:
                            nc.scalar.copy(recb[:], rec[:])
                        a = npool.tile([D, 512], BF16, tag="acc_bft")
                        nc.scalar.copy(a[:], acc[0:D, sl])
                        accbf.append(a[:])
                        recbs.append(recb[:])
                return (h, qh, accbf, recbs)

            def finish_norm(h, qh, accbf, recbs):
                """Scale by the reciprocal row (read partition-broadcast, so
                no PE ones-matmul is needed) and stack into the K=128-packed
                otn layout. Emitted a few kb into the following pass."""
                with nc.named_scope(f"norm{h}_{qh}"):
                    for j in range(2):
                        sl = slice(j * 512, (j + 1) * 512)
                        # (gpsimd partition_broadcast here measured WORSE:
                        # its queue latency in this chain cascades into pass-
                        # boundary stalls; the PE ones-matmul stays)
                        bc = pp.tile([128, 512], F32, tag="pp")
                        nc.tensor.matmul(
                            bc[0:D, :], ones_sb[:, 0:D], recbs[j],
                            start=True, stop=True,
                        )
                        rbb = bc[0:D, :]
                        if h == 0:
                            # rows 0..95 land on the same partitions: write the
                            # stack tile directly, no shift DMA needed
                            nc.vector.tensor_tensor(
                                otn[qh][0][0:D, sl], accbf[j], rbb, MULT
                            )
                            continue
                        ot = npool.tile([D, 512], BF16, tag="ot", bufs=4)
                        nc.vector.tensor_tensor(ot[:], accbf[j], rbb, MULT)
                        # stack rows 96h..96h+95 into the K=128-packed layout
                        # (DMAs alternate queues so the tail stacks land fast)
                        r0 = D * h
                        g0, off = divmod(r0, 128)
                        n0 = min(128 - off, D)
                        e1, e2 = (nc.sync, nc.scalar) if j == 0 else (nc.scalar, nc.sync)
                        e1.dma_start(otn[qh][g0][off:off + n0, sl], ot[0:n0, :])
                        if n0 < D:
                            e2.dma_start(
                                otn[qh][g0 + 1][0:D - n0, sl], ot[n0:D, :]
                            )

            def emit_out(nb, slot):
                qh, col = divmod(nb, 8)
                col *= 128
                with nc.named_scope(f"out{nb}"):
                    AB = None
                    if slot == "pp":
                        pa = pp.tile([128, 512], F32, tag="pp")
                        pb = pp.tile([128, 512], F32, tag="pp")
                        A, Bv = pa[:, 0:512], pb[:, 0:256]
                    elif slot == "acc":
                        t = pattn.tile([128, 1024], F32, tag="acc", bufs=1)
                        A, Bv, AB = t[:, 0:512], t[:, 512:768], t[:, 0:768]
                    else:
                        t = pattn.tile([128, 1024], F32, tag="et", bufs=2)
                        A, Bv, AB = t[:, 0:512], t[:, 512:768], t[:, 0:768]
                    for g in range(3):
                        nc.tensor.matmul(
                            A,
                            otn[qh][g][:, col:col + 128],
                            wp_sb[g][:, 0:512],
                            start=(g == 0),
                            stop=(g == 2),
                        )
                    for g in range(3):
                        nc.tensor.matmul(
                            Bv,
                            otn[qh][g][:, col:col + 128],
                            wp_sb[g][:, 512:768],
                            start=(g == 0),
                            stop=(g == 2),
                        )
                    osb = npool.tile([128, E], BF16, tag="osb", bufs=4)
                    if AB is not None:
                        # both psum halves live in one tile: one 768-wide add
                        nc.vector.tensor_tensor(osb[:], AB, bpb_sb[:], ADD)
                    else:
                        nc.vector.tensor_tensor(osb[:, 0:512], A, bpb_sb[:, 0:512], ADD)
                        nc.vector.tensor_tensor(
                            osb[:, 512:768], Bv, bpb_sb[:, 512:768], ADD
                        )
                    # spread across three queues so the tail transfers drain
                    # in parallel (exec time runs to the last DMA completion)
                    eng = [nc.sync, nc.scalar, nc.gpsimd][nb % 3]
                    eng.dma_start(out_d[nb * 128:(nb + 1) * 128, :], osb[:])

            # ---- per-head attention; qkproj of h+1 spread over both passes ----
            pending_norm = None
            for h in range(HL):
                qT, kT = tiles
                nxt = None
                for qh in range(2):
                    with nc.named_scope(f"attn{h}_{qh}"):
                        acc = pattn.tile([128, 1024], F32, tag="acc")

                        def emit_pv(kbp, pt):
                            for j in range(2):
                                nc.tensor.matmul(
                                    acc[0:DH, j * 512:(j + 1) * 512],
                                    vhat[kbp][:, h * DH:(h + 1) * DH],
                                    pt[:, j * 512:(j + 1) * 512],
                                    start=(kbp == 0),
                                    stop=(kbp == NB - 1),
                                )

                        pts = []
                        for kb in range(NB):
                            # exp stays one [128, 1024] op: splitting it into
                            # halves costs ~80ns/op of ACT overhead (+19us of
                            # exp time overall — measured regression)
                            et = pattn.tile([128, 1024], F32, tag="et", bufs=2)
                            if SCORE_MODE == "dr_resid":
                                kTb = kT[:, kb * 128:(kb + 1) * 128].unsqueeze(
                                    1
                                ).broadcast_to((D, 2, 128))
                            else:
                                kTb = kT[:, kb * 128:(kb + 1) * 128]
                            for j in range(2):
                                c = 2 * qh + j
                                if SCORE_MODE == "dr_resid":
                                    rhs = qT[:, :, c * 512:(c + 1) * 512]
                                else:
                                    rhs = qT[:, 0, c * 512:(c + 1) * 512]
                                nc.tensor.matmul(
                                    et[:, j * 512:(j + 1) * 512],
                                    kTb,
                                    rhs,
                                    start=True,
                                    stop=True,
                                    perf_mode=SCORE_PERF,
                                )
                            if kb >= LAG:
                                emit_pv(kb - LAG, pts[kb - LAG])
                            pt = ptpool.tile([128, 1024], BF16, tag="pt")
                            nc.scalar.activation(pt[:], et[:], AF.Exp, scale=SCALE)
                            pts.append(pt)
                            fin_kb = 3 if (h == 3 and qh == 1) else LAG
                            if kb == fin_kb and pending_norm is not None:
                                # previous pass's normalization: by now its
                                # reciprocal chain is done, so the broadcast
                                # matmuls won't stall the PE queue. One step
                                # earlier in the last pass so the otn stacks
                                # land before the first interleaved out block.
                                finish_norm(*pending_norm)
                                pending_norm = None
                            if h == 1 and qh == 0 and kb == 6:
                                load_wp()
                            g = qh * 16 + kb
                            if h + 1 < HL and g % 4 == 0:
                                # h3's k1..k3 chunks are deferred into its own
                                # first pass (which has no next-head projection
                                # and would idle); the remaining five h3 chunks
                                # spread 3+2 over h2's PE-bound passes.
                                if g == 0:
                                    nxt = start_qkproj(h + 1)
                                if h == 2:
                                    idx = {0: 0, 4: 1, 8: 2, 16: 3, 20: 4}.get(g)
                                    if idx is not None:
                                        emit_qkproj_chunk(3, nxt, idx)
                                else:
                                    emit_qkproj_chunk(h + 1, nxt, g // 4)
                            if h == 3 and qh == 0 and kb in (0, 1, 2):
                                # bunched at kb0-2: covers the PE idle window
                                # while ACT drains the previous pass's exps
                                emit_qkproj_chunk(3, (qT, kT), 5 + kb)
                            if h == 0 and qh == 0 and kb in (1, 5, 9, 11):
                                # head-0 work deferred out of the bw-bound
                                # startup: k2/k3 ready well before their
                                # scores (kb 8/12), q2/q3 before attn0_1
                                emit_qkproj_chunk(
                                    0, (qT, kT), {1: 6, 5: 7, 9: 2, 11: 3}[kb]
                                )
                            if h == 3 and qh == 1 and kb >= 5 and kb % 2 == 1:
                                emit_out((kb - 5) // 2, "pp")
                        for kbp in range(NB - LAG, NB):
                            emit_pv(kbp, pts[kbp])
                        pending_norm = drain_norm(h, qh, acc, tail=(h == HL - 1 and qh == 1))
                tiles = nxt

            # ---- output projection tail ----
            # nb 6/7 only need the qh=0 stacks: they keep the PE busy while
            # the last norm's reciprocal chain runs
            emit_out(6, "pp")
            finish_norm(*pending_norm)
            emit_out(7, "et")
            # 4-deep PSUM slot rotation (et alternates its two buffers) so
            # slot recycling isn't gated on the tail's busy DVE
            for nb, slot in zip(range(8, NB), ("acc", "et", "pp", "et", "acc", "et", "pp", "et")):
                emit_out(nb, slot)

    nc.compile()
    return nc


def _shard(x, Wqkv, bqkv, Wproj, bproj):
    """Build per-core input maps. Core c -> (batch c//2, head-group c%2)."""
    import ml_dtypes

    bf16 = ml_dtypes.bfloat16
    Wr = np.ascontiguousarray(Wqkv.reshape(E, H, D, 3))
    br = np.ascontiguousarray(bqkv.reshape(H, D, 3))
    ones = np.ones((1, 128), bf16)
    in_maps = []
    for c in range(NC):
        bb, hg = divmod(c, 2)
        hs = slice(hg * HL, (hg + 1) * HL)
        wq = np.ascontiguousarray(Wr[:, hs, :, 0].reshape(E, EL)).astype(bf16)
        wk = np.ascontiguousarray(Wr[:, hs, :, 1].reshape(E, EL)).astype(bf16)
        wv = np.zeros((E, HL, DH), np.float32)
        wv[:, :, :D] = Wr[:, hs, :, 2]
        bq = np.ascontiguousarray(br[hs, :, 0].T)  # [D, HL] (scale applied at exp)
        bk = np.ascontiguousarray(br[hs, :, 1].T)
        bv = np.zeros((HL, DH), np.float32)
        bv[:, :D] = br[hs, :, 2]
        bv[:, D] = 1.0  # denominator ones column
        wp = np.ascontiguousarray(Wproj[hg * EL:(hg + 1) * EL, :]).astype(bf16)
        bp = bproj if hg == 0 else np.zeros_like(bproj)
        in_maps.append({
            "wvxt": np.ascontiguousarray(
                np.concatenate([wv.reshape(E, HL * DH), x[bb].T], axis=1)
            ).astype(bf16),
            "wq": wq,
            "wk": wk,
            "bq": np.ascontiguousarray(bq, dtype=np.float32),
            "bk": np.ascontiguousarray(bk, dtype=np.float32),
            "bv": np.ascontiguousarray(bv.reshape(1, HL * DH)).astype(bf16),
            "wp": wp,
            "bp": np.ascontiguousarray(bp.reshape(1, E)).astype(bf16),
            "ones": ones,
            "ones32": np.ones((1, 128), np.float32),
        })
    return in_maps


def kernel(x, Wqkv, bqkv, Wproj, bproj):
    global _COMPILED, LAST_EXEC_NS, LAST_RESULTS
    x = np.asarray(x, dtype=np.float32)
    Wqkv = np.asarray(Wqkv, dtype=np.float32)
    bqkv = np.asarray(bqkv, dtype=np.float32)
    Wproj = np.asarray(Wproj, dtype=np.float32)
    bproj = np.asarray(bproj, dtype=np.float32)

    if _COMPILED is None:
        _COMPILED = _build()
    nc = _COMPILED

    in_maps = _shard(x, Wqkv, bqkv, Wproj, bproj)
    trace = bool(int(os.environ.get("BASS_MHA_TRACE", "0")))
    try:
        res = run_bass_kernel_spmd(nc, in_maps, list(range(NC)), trace=trace)
    except Exception:
        _device_reset()
        res = run_bass_kernel_spmd(nc, in_maps, list(range(NC)), trace=trace)
    LAST_EXEC_NS = res.exec_time_ns
    LAST_RESULTS = res

    out = np.empty((B, N, E), np.float32)
    for bb in range(B):
        out[bb] = res.results[2 * bb]["out"].astype(np.float32) + res.results[
            2 * bb + 1
        ]["out"].astype(np.float32)
    return out



# revision 80
# speedup vs baseline: 1.2929x; 1.0015x over previous
"""Multi-head attention TRN2 kernel (b=4, n=2048, e=768, h=8 heads, d=96).

Sharding: 8 cores = 4 batches x 2 head-groups (4 heads each).
Each core computes, for its (batch, head-group):
    qkv projection (its heads' columns of Wqkv), per-head attention
    (softmax over full n=2048), and a partial output projection
    (its heads' rows of Wproj). Host sums the two bf16 partial outputs
    per batch in f32 (row-parallel linear unshard) and concatenates.

All matmul operands are bf16 (fp32 PSUM accumulation): the TRN2 PE
streams exactly one moving column per cycle for every dtype (fp8
DoubleRow/DoubleColumn only extend the contraction depth, measured on
hw), so bf16 is already at peak column rate and fp8 would only add
quantization error. Scores are computed transposed (ET[nk, nq]) so no
transposes are needed; 1/sqrt(e) is folded into the exp activation's
scale; softmax denominators come from an extra ones-column appended to V
(row 96 of the PV accumulator). exp() skips max-subtraction: scaled
logits are bounded (~|2|). Per-head normalized outputs stay in SBUF and
are restacked to a K=128-packed [3 x 128, n] layout with SBUF->SBUF
DMAs; the output projection then runs 3 matmuls per chunk.

Schedule notes (all measured on hw):
- PV matmuls run LAG kb-steps behind exp; the LAG-deep PV tail covers
  the ACT exp backlog at pass boundaries. The norm drain uses DVE only
  (an ACT op there delays the next pass's exps and starves the PE of
  et buffers).
- The startup is HBM-bandwidth/latency-bound: only vproj c0-c2 and
  q0/q1/k0/k1 of head 0 run up front; head-0's q2/q3/k2/k3 defer into
  attn0_0. wp loads defer to mid-kernel; wk rides gpsimd behind xT c1.
- h3's k1-k3 chunks are emitted in its own first pass (bunched at kb
  0-2 to cover the boundary backlog); the remaining h3 chunks spread
  3+2 over h2's passes.
- out blocks 0-5 interleave into the last pass (stacks ready early via
  finish_norm at kb3); the tail rotates out8-15 over four PSUM slots.
"""

import os

import numpy as np

import concourse.bacc as bacc
import concourse.mybir as mybir
import concourse.tile as tile
from concourse.bass_utils import run_bass_kernel_spmd

B, N, E = 4, 2048, 768
H = 8          # total heads
HL = 4         # heads per core
D = E // H     # 96
DH = D + 1     # 97 (with denominator column)
KB = E // 128  # 6 contraction blocks
NB = N // 128  # 16 row blocks
NC = 8         # cores
EL = HL * D    # 384 local e-dim
SCALE = float(E) ** -0.5
LAG = 5        # PV runs LAG kb-steps behind exp

F32 = mybir.dt.float32
BF16 = mybir.dt.bfloat16
F8 = mybir.dt.float8e4
DR = mybir.MatmulPerfMode.DoubleRow
# 'bf16': plain bf16 scores (best error, PE col rate is 1/cycle regardless).
# 'dr_resid': fp8 DoubleRow, q as (hi, lo) residual pair — same speed as bf16.
# 'dc_plain': fp8 DoubleColumn, q/k plain fp8 — measured same speed as bf16.
SCORE_MODE = os.environ.get("SCORE_MODE", "bf16")
SCORE_PERF = {
    "bf16": None,
    "dr_resid": DR,
    "dc_plain": mybir.MatmulPerfMode.DoubleColumn,
}[SCORE_MODE]
QKDT = BF16 if SCORE_MODE == "bf16" else F8
AF = mybir.ActivationFunctionType
MULT = mybir.AluOpType.mult
ADD = mybir.AluOpType.add
SUB = mybir.AluOpType.subtract

_COMPILED = None
LAST_EXEC_NS = None
LAST_RESULTS = None


def _device_reset():
    """Recover a wedged NeuronCore (NRT_EXEC_UNIT_UNRECOVERABLE) via axon."""
    try:
        import ctypes
        import time

        import jax

        jax.devices()
        lib = ctypes.CDLL("/opt/axon/libaxon_pjrt.so")
        lib.axon_reset.restype = ctypes.c_int64
        lib.axon_reset()
        time.sleep(3)
    except Exception:
        pass


def _build():
    nc = bacc.Bacc("TRN2", target_bir_lowering=False, debug=False)

    wvxt_d = nc.dram_tensor("wvxt", [E, HL * DH + N], BF16, kind="ExternalInput")
    wq_d = nc.dram_tensor("wq", [E, EL], BF16, kind="ExternalInput")
    wk_d = nc.dram_tensor("wk", [E, EL], BF16, kind="ExternalInput")
    bq_d = nc.dram_tensor("bq", [D, HL], F32, kind="ExternalInput")
    bk_d = nc.dram_tensor("bk", [D, HL], F32, kind="ExternalInput")
    bv_d = nc.dram_tensor("bv", [1, HL * DH], BF16, kind="ExternalInput")
    wp_d = nc.dram_tensor("wp", [EL, E], BF16, kind="ExternalInput")
    bp_d = nc.dram_tensor("bp", [1, E], BF16, kind="ExternalInput")
    ones_d = nc.dram_tensor("ones", [1, 128], BF16, kind="ExternalInput")
    ones32_d = nc.dram_tensor("ones32", [1, 128], F32, kind="ExternalInput")
    out_d = nc.dram_tensor("out", [N, E], BF16, kind="ExternalOutput")

    with tile.TileContext(nc) as tc:
        with (
            tc.tile_pool(name="const", bufs=1) as cpool,
            tc.tile_pool(name="xt", bufs=1) as xpool,
            tc.tile_pool(name="qk", bufs=2) as qkpool,
            tc.tile_pool(name="vh", bufs=1) as vpool,
            tc.tile_pool(name="pt", bufs=LAG + 3) as ptpool,
            tc.tile_pool(name="nrm", bufs=3) as npool,
            tc.tile_pool(name="on", bufs=1) as opool,
            tc.tile_pool(name="pp", bufs=2, space="PSUM") as pp,
            tc.tile_pool(name="pattn", bufs=1, space="PSUM") as pattn,
        ):
            # ---- constants (DMA order matters: vproj prereqs first) ----
            ones_sb = cpool.tile([1, 128], BF16, tag="ones")
            nc.sync.dma_start(ones_sb[:], ones_d[:])
            bv_sb = cpool.tile([1, HL * DH], BF16, tag="bv")
            nc.sync.dma_start(bv_sb[:], bv_d[:])

            # Each kb row-block's wv and xT live in ONE packed tile
            # ("wvxt", wv cols 0:388 then xT cols 388:388+N): the first vproj
            # chain then needs just SIX transfers (two per queue) instead of
            # twelve, and per-queue completion spacing gates its start.
            WX = HL * DH
            xT_sb = []
            for kb in range(KB):
                t = xpool.tile([128, WX + N], BF16, tag=f"xt{kb}", name=f"xt{kb}")
                xT_sb.append(t)
            wv_sb = xT_sb  # wv operand is cols 0:WX of the packed tile

            # queue pairs (kb0/kb1 gpsimd, kb2/kb3 scalar, kb4/kb5 sync) give
            # arrival order ~[0,2,4,1,3,5]; the first vproj chains accumulate
            # in that order so the PE never waits a queue's second transfer
            VPROJ_KB = (0, 2, 4, 1, 3, 5)
            for kb, eng in zip(
                range(KB),
                [nc.gpsimd, nc.gpsimd, nc.scalar, nc.scalar, nc.sync, nc.sync],
            ):
                eng.dma_start(
                    xT_sb[kb][:, 0:WX + 512],
                    wvxt_d[kb * 128:(kb + 1) * 128, 0:WX + 512],
                )

            def xt_dma(eng, kb, c):
                eng.dma_start(
                    xT_sb[kb][:, WX + c * 512:WX + (c + 1) * 512],
                    wvxt_d[kb * 128:(kb + 1) * 128, WX + c * 512:WX + (c + 1) * 512],
                )

            # load the exp ACT table set off the critical path
            scr = npool.tile([1, 16], F32, tag="scr", bufs=1)
            nc.scalar.activation(scr[:], ones_sb[:, 0:16], AF.Exp)
            bq_sb = cpool.tile([D, HL], F32, tag="bq")
            nc.sync.dma_start(bq_sb[:], bq_d[:])
            bk_sb = cpool.tile([D, HL], F32, tag="bk")
            nc.sync.dma_start(bk_sb[:], bk_d[:])
            bp_sb = cpool.tile([1, E], BF16, tag="bp")
            nc.sync.dma_start(bp_sb[:], bp_d[:])

            # wq on sync (q-chunks run first and need only xT c0 + wq); wk on
            # gpsimd AFTER xT c1 so its 0.59MB doesn't compete for HBM during
            # the bandwidth-bound first ~8us (k0 isn't consumed until ~+16us)
            wq_sb = []
            wk_sb = []
            for kb in range(KB):
                t = cpool.tile([128, EL], BF16, tag=f"wq{kb}")
                nc.sync.dma_start(t[:], wq_d[kb * 128:(kb + 1) * 128, :])
                wq_sb.append(t)
            for kb in range(KB):
                xt_dma(nc.gpsimd, kb, 1)
            for kb in range(KB):
                t = cpool.tile([128, EL], BF16, tag=f"wk{kb}")
                nc.gpsimd.dma_start(t[:], wk_d[kb * 128:(kb + 1) * 128, :])
                wk_sb.append(t)
            for c in (2, 3):
                for kb in range(KB):
                    xt_dma(nc.gpsimd, kb, c)
            # wp loads are deferred into the head-1 attention pass: they are
            # not needed until the output projection (~80% through) and would
            # steal startup HBM bandwidth from xT/wq/wk.
            wp_sb = []

            def load_wp():
                for g in range(3):
                    t = cpool.tile([128, E], BF16, tag=f"wp{g}")
                    nc.gpsimd.dma_start(t[:], wp_d[g * 128:(g + 1) * 128, :])
                    wp_sb.append(t)

            # broadcast bias tiles (one K=1 matmul each, reused everywhere)
            bvb_sb = cpool.tile([128, HL * DH], BF16, tag="bvb")
            ps = pp.tile([128, 512], F32, tag="pp")
            nc.tensor.matmul(ps[:, 0:HL * DH], ones_sb[:], bv_sb[:], start=True, stop=True)
            nc.vector.tensor_copy(bvb_sb[:], ps[:, 0:HL * DH])
            bpb_sb = cpool.tile([128, E], BF16, tag="bpb")

            # normalized per-head outputs, heads stacked along partitions:
            # otn[qh][g][128g + r, i] = OT_(r//96)[r % 96, qh*1024 + i]
            otn = [
                [
                    opool.tile(
                        [128, 1024], BF16, tag=f"otn{qh}_{g}", name=f"otn{qh}_{g}"
                    )
                    for g in range(3)
                ]
                for qh in range(2)
            ]

            def start_qkproj(h):
                # q is kept as an fp8 (hi, lo) residual pair so the DoubleRow
                # scores matmul sees q at ~bf16 precision; k is quantized once
                # to fp8 (its pair dim is a stride-0 broadcast in the matmul).
                with nc.named_scope(f"qkproj{h}"):
                    nq = 1 if SCORE_MODE != "dr_resid" else 2
                    qT = qkpool.tile([D, nq, N], QKDT, tag="qT", name=f"qT{h}")
                    kT = qkpool.tile([D, N], QKDT, tag="kT", name=f"kT{h}")
                return (qT, kT)

            def emit_qkproj_chunk(h, tiles, i):
                qT, kT = tiles
                qk, c = divmod(i, 4)
                w_sb, b_sb = [(wq_sb, bq_sb), (wk_sb, bk_sb)][qk]
                sl = slice(c * 512, (c + 1) * 512)
                xsl = slice(WX + c * 512, WX + (c + 1) * 512)
                with nc.named_scope(f"qkproj{h}"):
                    ps = pp.tile([128, 512], F32, tag="pp", name=f"psqk{h}_{i}")
                    for kb in range(KB):
                        nc.tensor.matmul(
                            ps[0:D, :],
                            w_sb[kb][:, h * D:(h + 1) * D],
                            xT_sb[kb][:, xsl],
                            start=(kb == 0),
                            stop=(kb == KB - 1),
                        )
                    if qk == 1:
                        nc.vector.tensor_scalar(
                            kT[:, sl], ps[0:D, :], 1.0, b_sb[:, h:h + 1], MULT, ADD
                        )
                    elif SCORE_MODE == "dr_resid":
                        qtmp = npool.tile([D, 512], BF16, tag="qtmp", bufs=2)
                        nc.vector.tensor_scalar(
                            qtmp[:], ps[0:D, :], 1.0, b_sb[:, h:h + 1], MULT, ADD
                        )
                        nc.vector.tensor_copy(qT[:, 0, sl], qtmp[:])
                        nc.vector.tensor_tensor(
                            qT[:, 1, sl], qtmp[:], qT[:, 0, sl], SUB
                        )
                    else:
                        nc.vector.tensor_scalar(
                            qT[:, 0, sl], ps[0:D, :], 1.0, b_sb[:, h:h + 1],
                            MULT, ADD,
                        )

            # ---- V-hat projection interleaved with head-0 qk projection ----
            # Only vproj c0-c2, q0, q1 and k0 run up front: the startup region
            # is HBM-bandwidth-bound, so the rest of head-0's chunks and the
            # vproj c3 block group are deferred into the attn0_0 pass.
            vhat = []

            def emit_vproj_block(nb):
                with nc.named_scope("vproj"):
                    ps = pp.tile([128, 512], F32, tag="pp")
                    for i, kb in enumerate(VPROJ_KB):
                        nc.tensor.matmul(
                            ps[:, 0:HL * DH],
                            xT_sb[kb][:, WX + nb * 128:WX + (nb + 1) * 128],
                            wv_sb[kb][:, 0:WX],
                            start=(i == 0),
                            stop=(i == KB - 1),
                        )
                    vt = vpool.tile([128, HL * DH], BF16, tag=f"vh{nb}")
                    nc.vector.tensor_tensor(vt[:], ps[:, 0:HL * DH], bvb_sb[:], ADD)
                    vhat.append(vt)

            tiles = start_qkproj(0)
            for c in range(4):
                for nb in range(4 * c, 4 * c + 4):
                    emit_vproj_block(nb)
                # q0, q1, k0, k1 up front; q2/q3/k2/k3 defer into attn0_0
                emit_qkproj_chunk(0, tiles, [0, 1, 4, 5][c])
                if c == 1:
                    # build the bproj broadcast (off the critical start) in a
                    # free attention-et PSUM slot (one per bank half) so it
                    # doesn't contend with the qkproj chains' pp slots
                    ps = pattn.tile([128, 1024], F32, tag="et", bufs=2)
                    for off, w in [(0, 512), (512, 256)]:
                        nc.tensor.matmul(
                            ps[:, off:off + w], ones_sb[:], bp_sb[:, off:off + w],
                            start=True, stop=True,
                        )
                    nc.vector.tensor_copy(bpb_sb[:], ps[:, 0:E])

            def drain_norm(h, qh, acc, tail=False):
                """Drain the PSUM accumulator (bf16 rows + f32 denominator
                row) so the next pass's PV(0) gets the buffer back fast, and
                run the reciprocal chains (DVE only — ACT still has an exp
                backlog at the pass boundary, so any ACT op here delays the
                next pass's exps and starves the PE of et buffers). Tail:
                per-j chains with ACT offload so the j=0 half unblocks the
                output projection early."""
                with nc.named_scope(f"norm{h}_{qh}"):
                    if not tail:
                        a = npool.tile([D, 1024], BF16, tag="acc_bf")
                        nc.vector.tensor_copy(a[:], acc[0:D, :])
                        sums = npool.tile([1, 1024], F32, tag="sums")
                        nc.vector.tensor_copy(sums[:], acc[D:DH, :])
                        rec = npool.tile([1, 1024], F32, tag="rec")
                        nc.vector.reciprocal_approx_fast(rec[:], sums[:])
                        recb = npool.tile([1, 1024], BF16, tag="recb")
                        nc.vector.tensor_copy(recb[:], rec[:])
                        accbf = [a[:, 0:512], a[:, 512:1024]]
                        recbs = [recb[:, 0:512], recb[:, 512:1024]]
                        return (h, qh, accbf, recbs)
                    accbf, recbs = [], []
                    for j in range(2):
                        sl = slice(j * 512, (j + 1) * 512)
                        sums = npool.tile([1, 512], F32, tag="sumst")
                        if j == 0:
                            nc.vector.tensor_copy(sums[:], acc[D:DH, sl])
                        else:
                            nc.scalar.copy(sums[:], acc[D:DH, sl])
                        rec = npool.tile([1, 512], F32, tag="rect")
                        nc.vector.reciprocal_approx_fast(rec[:], sums[:])
                        recb = npool.tile([1, 512], BF16, tag="recbt")
                        if j == 0:
                            nc.vector.tensor_copy(recb[:], rec[:])
                        else:
                            nc.scalar.copy(recb[:], rec[:])
                        a = npool.tile([D, 512], BF16, tag="acc_bft")
                        nc.scalar.copy(a[:], acc[0:D, sl])
                        accbf.append(a[:])
                        recbs.append(recb[:])
                return (h, qh, accbf, recbs)

            def finish_norm(h, qh, accbf, recbs):
                """Scale by the reciprocal row (read partition-broadcast, so
                no PE ones-matmul is needed) and stack into the K=128-packed
                otn layout. Emitted a few kb into the following pass."""
                with nc.named_scope(f"norm{h}_{qh}"):
                    for j in range(2):
                        sl = slice(j * 512, (j + 1) * 512)
                        # (gpsimd partition_broadcast here measured WORSE:
                        # its queue latency in this chain cascades into pass-
                        # boundary stalls; the PE ones-matmul stays)
                        bc = pp.tile([128, 512], F32, tag="pp")
                        nc.tensor.matmul(
                            bc[0:D, :], ones_sb[:, 0:D], recbs[j],
                            start=True, stop=True,
                        )
                        rbb = bc[0:D, :]
                        if h == 0:
                            # rows 0..95 land on the same partitions: write the
                            # stack tile directly, no shift DMA needed
                            nc.vector.tensor_tensor(
                                otn[qh][0][0:D, sl], accbf[j], rbb, MULT
                            )
                            continue
                        ot = npool.tile([D, 512], BF16, tag="ot", bufs=4)
                        nc.vector.tensor_tensor(ot[:], accbf[j], rbb, MULT)
                        # stack rows 96h..96h+95 into the K=128-packed layout
                        # (DMAs alternate queues so the tail stacks land fast)
                        r0 = D * h
                        g0, off = divmod(r0, 128)
                        n0 = min(128 - off, D)
                        e1, e2 = (nc.sync, nc.scalar) if j == 0 else (nc.scalar, nc.sync)
                        e1.dma_start(otn[qh][g0][off:off + n0, sl], ot[0:n0, :])
                        if n0 < D:
                            e2.dma_start(
                                otn[qh][g0 + 1][0:D - n0, sl], ot[n0:D, :]
                            )

            def emit_out(nb, slot):
                qh, col = divmod(nb, 8)
                col *= 128
                with nc.named_scope(f"out{nb}"):
                    AB = None
                    if slot == "pp":
                        pa = pp.tile([128, 512], F32, tag="pp")
                        pb = pp.tile([128, 512], F32, tag="pp")
                        A, Bv = pa[:, 0:512], pb[:, 0:256]
                    elif slot == "acc":
                        t = pattn.tile([128, 1024], F32, tag="acc", bufs=1)
                        A, Bv, AB = t[:, 0:512], t[:, 512:768], t[:, 0:768]
                    else:
                        t = pattn.tile([128, 1024], F32, tag="et", bufs=2)
                        A, Bv, AB = t[:, 0:512], t[:, 512:768], t[:, 0:768]
                    for g in range(3):
                        nc.tensor.matmul(
                            A,
                            otn[qh][g][:, col:col + 128],
                            wp_sb[g][:, 0:512],
                            start=(g == 0),
                            stop=(g == 2),
                        )
                    for g in range(3):
                        nc.tensor.matmul(
                            Bv,
                            otn[qh][g][:, col:col + 128],
                            wp_sb[g][:, 512:768],
                            start=(g == 0),
                            stop=(g == 2),
                        )
                    osb = npool.tile([128, E], BF16, tag="osb", bufs=4)
                    if AB is not None:
                        # both psum halves live in one tile: one 768-wide add
                        nc.vector.tensor_tensor(osb[:], AB, bpb_sb[:], ADD)
                    else:
                        nc.vector.tensor_tensor(osb[:, 0:512], A, bpb_sb[:, 0:512], ADD)
                        nc.vector.tensor_tensor(
                            osb[:, 512:768], Bv, bpb_sb[:, 512:768], ADD
                        )
                    # spread across three queues so the tail transfers drain
                    # in parallel (exec time runs to the last DMA completion)
                    eng = [nc.sync, nc.scalar, nc.gpsimd][nb % 3]
                    eng.dma_start(out_d[nb * 128:(nb + 1) * 128, :], osb[:])

            # ---- per-head attention; qkproj of h+1 spread over both passes ----
            pending_norm = None
            for h in range(HL):
                qT, kT = tiles
                nxt = None
                for qh in range(2):
                    with nc.named_scope(f"attn{h}_{qh}"):
                        acc = pattn.tile([128, 1024], F32, tag="acc")

                        def emit_pv(kbp, pt):
                            for j in range(2):
                                nc.tensor.matmul(
                                    acc[0:DH, j * 512:(j + 1) * 512],
                                    vhat[kbp][:, h * DH:(h + 1) * DH],
                                    pt[:, j * 512:(j + 1) * 512],
                                    start=(kbp == 0),
                                    stop=(kbp == NB - 1),
                                )

                        pts = []
                        for kb in range(NB):
                            # exp stays one [128, 1024] op: splitting it into
                            # halves costs ~80ns/op of ACT overhead (+19us of
                            # exp time overall — measured regression)
                            et = pattn.tile([128, 1024], F32, tag="et", bufs=2)
                            if SCORE_MODE == "dr_resid":
                                kTb = kT[:, kb * 128:(kb + 1) * 128].unsqueeze(
                                    1
                                ).broadcast_to((D, 2, 128))
                            else:
                                kTb = kT[:, kb * 128:(kb + 1) * 128]
                            for j in range(2):
                                c = 2 * qh + j
                                if SCORE_MODE == "dr_resid":
                                    rhs = qT[:, :, c * 512:(c + 1) * 512]
                                else:
                                    rhs = qT[:, 0, c * 512:(c + 1) * 512]
                                nc.tensor.matmul(
                                    et[:, j * 512:(j + 1) * 512],
                                    kTb,
                                    rhs,
                                    start=True,
                                    stop=True,
                                    perf_mode=SCORE_PERF,
                                )
                            if kb >= LAG:
                                emit_pv(kb - LAG, pts[kb - LAG])
                            pt = ptpool.tile([128, 1024], BF16, tag="pt")
                            nc.scalar.activation(pt[:], et[:], AF.Exp, scale=SCALE)
                            pts.append(pt)
                            fin_kb = 3 if (h == 3 and qh == 1) else LAG
                            if kb == fin_kb and pending_norm is not None:
                                # previous pass's normalization: by now its
                                # reciprocal chain is done, so the broadcast
                                # matmuls won't stall the PE queue. One step
                                # earlier in the last pass so the otn stacks
                                # land before the first interleaved out block.
                                finish_norm(*pending_norm)
                                pending_norm = None
                            if h == 1 and qh == 0 and kb == 6:
                                load_wp()
                            g = qh * 16 + kb
                            if h + 1 < HL and g % 4 == 0:
                                # h3's k1..k3 chunks are deferred into its own
                                # first pass (which has no next-head projection
                                # and would idle); the remaining five h3 chunks
                                # spread 3+2 over h2's PE-bound passes.
                                if g == 0:
                                    nxt = start_qkproj(h + 1)
                                if h == 2:
                                    idx = {0: 0, 4: 1, 8: 2, 16: 3, 20: 4}.get(g)
                                    if idx is not None:
                                        emit_qkproj_chunk(3, nxt, idx)
                                else:
                                    emit_qkproj_chunk(h + 1, nxt, g // 4)
                            if h == 3 and qh == 0 and kb in (0, 1, 2):
                                # bunched at kb0-2: covers the PE idle window
                                # while ACT drains the previous pass's exps
                                emit_qkproj_chunk(3, (qT, kT), 5 + kb)
                            if h == 0 and qh == 0 and kb in (1, 5, 9, 11):
                                # head-0 work deferred out of the bw-bound
                                # startup: k2/k3 ready well before their
                                # scores (kb 8/12), q2/q3 before attn0_1
                                emit_qkproj_chunk(
                                    0, (qT, kT), {1: 6, 5: 7, 9: 2, 11: 3}[kb]
                                )
                            if h == 3 and qh == 1 and kb >= 5 and kb % 2 == 1:
                                emit_out((kb - 5) // 2, "pp")
                        for kbp in range(NB - LAG, NB):
                            emit_pv(kbp, pts[kbp])
                        pending_norm = drain_norm(h, qh, acc, tail=(h == HL - 1 and qh == 1))
                tiles = nxt

            # ---- output projection tail ----
            # nb 6/7 only need the qh=0 stacks: they keep the PE busy while
            # the last norm's reciprocal chain runs
            emit_out(6, "pp")
            finish_norm(*pending_norm)
            emit_out(7, "et")
            # 4-deep PSUM slot rotation (et alternates its two buffers) so
            # slot recycling isn't gated on the tail's busy DVE
            for nb, slot in zip(range(8, NB), ("acc", "et", "pp", "et", "acc", "et", "pp", "et")):
                emit_out(nb, slot)

    nc.compile()
    return nc


def _shard(x, Wqkv, bqkv, Wproj, bproj):
    """Build per-core input maps. Core c -> (batch c//2, head-group c%2)."""
    import ml_dtypes

    bf16 = ml_dtypes.bfloat16
    Wr = np.ascontiguousarray(Wqkv.reshape(E, H, D, 3))
    br = np.ascontiguousarray(bqkv.reshape(H, D, 3))
    ones = np.ones((1, 128), bf16)
    in_maps = []
    for c in range(NC):
        bb, hg = divmod(c, 2)
        hs = slice(hg * HL, (hg + 1) * HL)
        wq = np.ascontiguousarray(Wr[:, hs, :, 0].reshape(E, EL)).astype(bf16)
        wk = np.ascontiguousarray(Wr[:, hs, :, 1].reshape(E, EL)).astype(bf16)
        wv = np.zeros((E, HL, DH), np.float32)
        wv[:, :, :D] = Wr[:, hs, :, 2]
        bq = np.ascontiguousarray(br[hs, :, 0].T)  # [D, HL] (scale applied at exp)
        bk = np.ascontiguousarray(br[hs, :, 1].T)
        bv = np.zeros((HL, DH), np.float32)
        bv[:, :D] = br[hs, :, 2]
        bv[:, D] = 1.0  # denominator ones column
        wp = np.ascontiguousarray(Wproj[hg * EL:(hg + 1) * EL, :]).astype(bf16)
        bp = bproj if hg == 0 else np.zeros_like(bproj)
        in_maps.append({
            "wvxt": np.ascontiguousarray(
                np.concatenate([wv.reshape(E, HL * DH), x[bb].T], axis=1)
            ).astype(bf16),
            "wq": wq,
            "wk": wk,
            "bq": np.ascontiguousarray(bq, dtype=np.float32),
            "bk": np.ascontiguousarray(bk, dtype=np.float32),
            "bv": np.ascontiguousarray(bv.reshape(1, HL * DH)).astype(bf16),
            "wp": wp,
            "bp": np.ascontiguousarray(bp.reshape(1, E)).astype(bf16),
            "ones": ones,
            "ones32": np.ones((1, 128), np.float32),
        })
    return in_maps


def kernel(x, Wqkv, bqkv, Wproj, bproj):
    global _COMPILED, LAST_EXEC_NS, LAST_RESULTS
    x = np.asarray(x, dtype=np.float32)
    Wqkv = np.asarray(Wqkv, dtype=np.float32)
    bqkv = np.asarray(bqkv, dtype=np.float32)
    Wproj = np.asarray(Wproj, dtype=np.float32)
    bproj = np.asarray(bproj, dtype=np.float32)

    if _COMPILED is None:
        _COMPILED = _build()
    nc = _COMPILED

    in_maps = _shard(x, Wqkv, bqkv, Wproj, bproj)
    trace = bool(int(os.environ.get("BASS_MHA_TRACE", "0")))
    try:
        res = run_bass_kernel_spmd(nc, in_maps, list(range(NC)), trace=trace)
    except Exception:
        _device_reset()
        res = run_bass_kernel_spmd(nc, in_maps, list(range(NC)), trace=trace)
    LAST_EXEC_NS = res.exec_time_ns
    LAST_RESULTS = res

    out = np.empty((B, N, E), np.float32)
    for bb in range(B):
        out[bb] = res.results[2 * bb]["out"].astype(np.float32) + res.results[
            2 * bb + 1
        ]["out"].astype(np.float32)
    return out

